# revision 36
# baseline (speedup 1.0000x reference)
"""Longformer decoder (4 layers, sliding-window causal attention) on 8 trn2 cores.

Sharding: 4096 tokens (B=2 x S=2048) split into 8 contiguous chunks of 512
(core = b*4 + chunk). Activations are kept d-major ([dim, token], dim on
partitions) so every matmul is weights-stationary with no transposes.

v2 structure (vs v1 baseline):
- LN affine (scale/bias) folded into the projection weights host-side; the
  kernel's LN emits the unaffined z=(x-mu)*rstd, with biases re-applied via
  scalar-engine Identity copies (per-partition bias) where needed.
- LN statistics matmuls run on f32r bitcasts of the residual directly (no
  gpsimd f16 staging copies).
- reciprocal_approx_fast for all softmax/LN reciprocals.
- Attention restructured per-head into kt (key-tile) granularity with f16
  masks; per layer, all heads' own-key work (qB=1) runs first so the x-halo
  AllGather from the previous layer is consumed ~40us into the layer.
- Weights double-buffered and prefetched one layer ahead; w_out cached in
  SBUF during the last layer.
- Final logits loop runs the core's own 512-token block first (directly off
  the local yf) while the 8-way AllGather is in flight; remaining blocks are
  imported with per-core indirect DMAs. Output token-blocks are rotated
  per-core ((c+j)%8) so the SPMD program stays identical; host unrotates.
- Logits written f16 (tolerance 2e-2; f16 quantization ~5e-4).
"""
import os
import sys

import numpy as np

for _p in ("/opt/trn_rl_repo", "/root/.axon_site/_ro/trn_rl_repo"):
    if os.path.isdir(_p) and _p not in sys.path:
        sys.path.insert(0, _p)

import concourse.bass as bass
import concourse.mybir as mybir
import concourse.tile as tile
from concourse import bacc
from concourse.bass import ts, ds
from concourse.bass_utils import run_bass_kernel_spmd
from concourse.masks import make_identity

F32 = mybir.dt.float32
F32R = mybir.dt.float32r
F16 = mybir.dt.float16
I32 = mybir.dt.int32
MDT = F16 if os.environ.get("KMMDT", "f16") == "f16" else F32R
AF = mybir.ActivationFunctionType
OP = mybir.AluOpType

B, S, V, D, H, NL, MLPD = 2, 2048, 32000, 512, 8, 4, 2048
DH = D // H            # 64
HALF = 256             # attention half-window (WIN // 2)
P = 128
NCORES = 8
CHUNK = 512            # own tokens per core
W = CHUNK + HALF       # 768 = halo + own
DT = D // P            # 4 d-tiles
MT = MLPD // P         # 16 mlp tiles
VSH = 4096             # padded vocab shard (actual 4000)
NTOK = B * S           # 4096
VV = V // NCORES       # 4000 valid vocab per core
GROUPS = [[0, 1, 2, 3], [4, 5, 6, 7]]
EXP_SHIFT = 2.0
SCALE = float(1.0 / np.sqrt(DH))
# key-tile table: (kt, q0, width) — q columns [q0, q0+width) see key tile kt
KTW = [(0, 0, 256), (1, 0, 256), (2, 0, 512), (3, 0, 512), (4, 256, 256), (5, 256, 256)]

_CACHE = {}


# ================================================================ builder
def _build():
    nc = bacc.Bacc("TRN2", target_bir_lowering=False, debug=False,
                   num_devices=NCORES)

    ein = lambda n, sh, dt=F32: nc.dram_tensor(n, sh, dt, kind="ExternalInput")
    io = dict(
        wq=ein("wq", [NL, D, D], MDT), wk=ein("wk", [NL, D, D], MDT),
        wv=ein("wv", [NL, D, D], MDT), wo=ein("wo", [NL, D, D], MDT),
        w1=ein("w1", [NL, D, MLPD], MDT), w2=ein("w2", [NL, MLPD, D], MDT),
        b1=ein("b1", [NL, MLPD]), b2=ein("b2", [NL, D]),
        bq=ein("bq", [NL, D]), bk=ein("bk", [NL, D]), bv=ein("bv", [P, NL, D], F16),
        w_out_sl=ein("w_out_sl", [D, V], MDT), b_out_sl=ein("b_out_sl", [1, V]),
        embed=ein("embed", [V, D]),
        idx_in=ein("idx_in", [P, W // P], I32),
        pe_dm=ein("pe_dm", [D, W]),
        masks=ein("masks", [6, P, 512], F16),
        halo_offs=ein("halo_offs", [P, DT], I32),
        out=nc.dram_tensor("logits_vm", [V, CHUNK], F16, kind="ExternalOutput"),
    )
    if os.environ.get("KDEBUG") == "1":
        io["xdump"] = nc.dram_tensor("xdump", [NL + 1, D, CHUNK], F32, kind="ExternalOutput")
        io["ydump"] = nc.dram_tensor("ydump", [D, W], MDT, kind="ExternalOutput")
        io["qdump"] = nc.dram_tensor("qdump", [D, CHUNK], MDT, kind="ExternalOutput")
        io["kdump"] = nc.dram_tensor("kdump", [D, W], MDT, kind="ExternalOutput")
        io["adump"] = nc.dram_tensor("adump", [D, CHUNK], MDT, kind="ExternalOutput")

    with tile.TileContext(nc) as tc, nc.allow_low_precision(reason="f32r rounding"):
        _emit(nc, tc, io)
    nc.compile()
    return nc


def _emit(nc, tc, io):
    cpool = tc.alloc_tile_pool(name="const", bufs=1)
    xpool = tc.alloc_tile_pool(name="xres", bufs=1)
    wqk = tc.alloc_tile_pool(name="wqkvo", bufs=2)
    wmlp = tc.alloc_tile_pool(name="wmlp", bufs=1)
    ps_a = tc.alloc_tile_pool(name="ps_a", bufs=2, space="PSUM")
    ps_b = tc.alloc_tile_pool(name="ps_b", bufs=4, space="PSUM")
    ps_c = tc.alloc_tile_pool(name="ps_c", bufs=2, space="PSUM")
    drp = tc.alloc_tile_pool(name="drbounce", bufs=1, space="DRAM")

    # ------------------------------------------------ constants
    ones_f = cpool.tile([P, P], F32, tag="ones_f")
    nc.vector.memset(ones_f[:], 1.0)
    ones = cpool.tile([P, P], MDT, tag="ones")
    nc.vector.tensor_copy(out=ones[:], in_=ones_f[:])
    ones_r = cpool.tile([P, P], F32R, tag="ones_r")
    nc.vector.tensor_copy(out=ones_r[:], in_=ones_f[:])
    negb = cpool.tile([P, 1], F32, tag="negb")
    nc.vector.memset(negb[:], EXP_SHIFT)
    epsb = cpool.tile([P, 1], F32, tag="epsb")
    nc.vector.memset(epsb[:], 1e-6)
    b1t = cpool.tile([P, NL, MT], F32, tag="b1t")
    b2t = cpool.tile([P, NL, DT], F32, tag="b2t")
    bqt = cpool.tile([P, NL, DT], F32, tag="bqt")
    bkt = cpool.tile([P, NL, DT], F32, tag="bkt")
    bvt = cpool.tile([P, NL, D], MDT, tag="bvt")
    bot = cpool.tile([P, V // P], F32, tag="bot")
    maskt = cpool.tile([P, 6, 512], F16, tag="maskt")
    hoffs = cpool.tile([P, DT], I32, tag="hoffs")

    # residual stream (own 512 tokens, d-major) + per-layer halo + final LN out.
    # F32R (same bits as f32) so the LN stat matmuls can consume x directly.
    x = xpool.tile([P, DT, CHUNK], F32R, tag="x")
    xh = xpool.tile([P, DT, HALF], F32R, tag="xh")
    yf = xpool.tile([P, DT, CHUNK], MDT, tag="yf")

    # ------------------------------------------------ weight loads
    def load_qkvo(l):
        w = {}
        for nm in ("wq", "wk", "wv", "wo"):
            t = wqk.tile([P, DT, D], MDT, tag=nm, name=nm)
            nc.sync.dma_start(out=t[:], in_=io[nm].ap()[l].rearrange("(t p) m -> p t m", p=P))
            w[nm] = t
        return w

    def load_mlp(l):
        w1r = wmlp.tile([P, DT, MLPD], MDT, tag="w1r")
        nc.sync.dma_start(out=w1r[:], in_=io["w1"].ap()[l].rearrange("(t p) m -> p t m", p=P))
        w2r = wmlp.tile([P, MT, D], MDT, tag="w2r")
        nc.sync.dma_start(out=w2r[:], in_=io["w2"].ap()[l].rearrange("(t p) m -> p t m", p=P))
        return w1r, w2r

    # ------------------------------------------------ embedding (queued FIRST
    # so the gather/transpose pipeline runs while weights stream in behind it)
    with tc.tile_pool(name="embed", bufs=1) as epool:
        ident = epool.tile([P, P], F32, tag="ident")
        make_identity(nc, ident[:])
        idxt = epool.tile([P, W // P], I32, tag="idxt")
        nc.sync.dma_start(out=idxt[:], in_=io["idx_in"].ap())
        pe = epool.tile([P, DT, W], F32, tag="pe")
        nc.sync.dma_start(out=pe[:], in_=io["pe_dm"].ap().rearrange("(t p) m -> p t m", p=P))
        nc.sync.dma_start(out=b1t[:], in_=io["b1"].ap().rearrange("l (m p) -> p l m", p=P))
        nc.sync.dma_start(out=b2t[:], in_=io["b2"].ap().rearrange("l (t p) -> p l t", p=P))
        nc.sync.dma_start(out=bqt[:], in_=io["bq"].ap().rearrange("l (t p) -> p l t", p=P))
        nc.sync.dma_start(out=bkt[:], in_=io["bk"].ap().rearrange("l (t p) -> p l t", p=P))
        nc.sync.dma_start(out=bvt[:], in_=io["bv"].ap())
        nc.sync.dma_start(out=bot[:], in_=io["b_out_sl"].ap().rearrange("o (t p) -> p (o t)", p=P))
        nc.sync.dma_start(out=maskt[:], in_=io["masks"].ap().rearrange("k p q -> p k q"))
        nc.sync.dma_start(out=hoffs[:], in_=io["halo_offs"].ap())
        wcur = load_qkvo(0)
        mcur = load_mlp(0)
        with tc.tile_pool(name="gath", bufs=2) as gpool:
            for g in range(W // P):
                gt = gpool.tile([P, D], F32, tag="gt")
                nc.gpsimd.indirect_dma_start(
                    out=gt[:], out_offset=None, in_=io["embed"].ap(),
                    in_offset=bass.IndirectOffsetOnAxis(ap=idxt[:, g:g + 1], axis=0),
                )
                for dt in range(DT):
                    pt = ps_a.tile([P, P], F32, tag="ps_a")
                    nc.tensor.transpose(pt[:], gt[:, ts(dt, P)], ident[:])
                    dst = xh[:, dt, ts(g, P)] if g < 2 else x[:, dt, ts(g - 2, P)]
                    nc.vector.tensor_add(out=dst, in0=pt[:], in1=pe[:, dt, ts(g, P)])

    def dump_x(slot):
        if "xdump" in io:
            nc.sync.dma_start(out=io["xdump"].ap()[slot].rearrange("(t p) m -> p t m", p=P),
                              in_=x[:, :, :].bitcast(F32))
    dump_x(0)

    # ------------------------------------------------ layer pools
    lp = tc.alloc_tile_pool(name="layers", bufs=1)
    lp3 = tc.alloc_tile_pool(name="ltrans3", bufs=3)

    def emit_ln(srcs, y):
        """LN over d (partition axis); emits z=(x-mu)*rstd (no affine).
        srcs: list of (fn(dt)->AP[128,width] F32, y_col0, width)."""
        srcs2 = []
        for fn, col0, width in srcs:
            o = 0
            while o < width:
                w = min(512, width - o)
                srcs2.append((lambda dt, fn=fn, o=o, w=w: fn(dt)[:, ds(o, w)], col0 + o, w))
                o += w
        for fn, col0, w in srcs2:
            sx = ps_c.tile([1, 512], F32, tag="ps_c")
            sxx = ps_c.tile([1, 512], F32, tag="ps_c")
            for dt in range(DT):
                xsq = lp3.tile([P, 512], MDT, tag="ln_xsq", bufs=2)
                nc.gpsimd.tensor_mul(out=xsq[:, :w], in0=fn(dt), in1=fn(dt))
                nc.tensor.matmul(out=sx[:, :w], lhsT=ones_r[:, 0:1], rhs=fn(dt),
                                 start=(dt == 0), stop=(dt == DT - 1))
                nc.tensor.matmul(out=sxx[:, :w], lhsT=ones[:, 0:1], rhs=xsq[:, :w],
                                 start=(dt == 0), stop=(dt == DT - 1))
            # murs: [mu | rstd] f16, built by the scalar engine (short chain):
            # mu = sx/D; mu2/var on DVE; rstd = rsqrt(var+eps) directly.
            murs = lp3.tile([1, 1024], MDT, tag="ln_murs", bufs=2)
            mu2 = lp3.tile([1, 512], F32, tag="ln_mu2", bufs=2)
            var = lp3.tile([1, 512], F32, tag="ln_var", bufs=2)
            nc.scalar.activation(murs[:, 0:w], sx[:, :w], AF.Identity, bias=0.0,
                                 scale=1.0 / D)
            nc.vector.tensor_mul(out=mu2[:, :w], in0=murs[:, 0:w], in1=murs[:, 0:w])
            nc.vector.scalar_tensor_tensor(
                out=var[:, :w], in0=sxx[:, :w], scalar=1.0 / D,
                in1=mu2[:, :w], op0=OP.mult, op1=OP.subtract)
            nc.scalar.activation(murs[:, 512:512 + w], var[:, :w],
                                 AF.Abs_reciprocal_sqrt, bias=epsb[0:1, :], scale=1.0)
            if w == 256:
                pmr = ps_a.tile([P, 512], F32, tag="ps_a")
                nc.tensor.matmul(out=pmr[:, 0:256], lhsT=ones[0:1, :],
                                 rhs=murs[:, 0:256], start=True, stop=True)
                nc.tensor.matmul(out=pmr[:, 256:512], lhsT=ones[0:1, :],
                                 rhs=murs[:, 512:768], start=True, stop=True)
                pmu, prs = pmr[:, 0:256], pmr[:, 256:512]
            else:
                pmu_t = ps_a.tile([P, 512], F32, tag="ps_a")
                nc.tensor.matmul(out=pmu_t[:], lhsT=ones[0:1, :], rhs=murs[:, 0:512],
                                 start=True, stop=True)
                prs_t = ps_a.tile([P, 512], F32, tag="ps_a")
                nc.tensor.matmul(out=prs_t[:], lhsT=ones[0:1, :], rhs=murs[:, 512:1024],
                                 start=True, stop=True)
                pmu, prs = pmu_t[:], prs_t[:]
            for dt in range(DT):
                scr = lp3.tile([P, 512], MDT, tag="ln_scr", bufs=2)
                nc.vector.tensor_sub(out=scr[:, :w], in0=fn(dt), in1=pmu)
                nc.vector.tensor_mul(out=y[:, dt, ds(col0, w)], in0=scr[:, :w], in1=prs)

    # ------------------------------------------------ transformer layers
    # Each layer is split into token-halves: the second half (tokens 256-511)
    # finishes first — through attention qB=1, O-proj, MLP, residual — and its
    # x is exported + AllGathered mid-layer. The gather-dependent first half
    # (halo LN, K/V halo, qB=0) of the NEXT layer then trails the export by a
    # full half-layer of independent work, hiding collective latency + skew.
    _knl = int(os.environ.get("KNL", NL))
    prev_agout = None
    for l in range(_knl):
        wq_r, wk_r, wv_r, wo_r = wcur["wq"], wcur["wk"], wcur["wv"], wcur["wo"]
        w1r, w2r = mcur

        y = lp.tile([P, DT, W], MDT, tag="y")
        # LN1 on own tokens (halo part deferred until the AllGather landed)
        emit_ln(srcs=[(lambda dt: x[:, dt, :], HALF, CHUNK)], y=y)

        # --- Q projection (own tokens only)
        qr = lp.tile([P, DT, CHUNK], MDT, tag="qr")
        for do in range(DT):
            pq = ps_a.tile([P, CHUNK], F32, tag="ps_a")
            for dt in range(DT):
                nc.tensor.matmul(out=pq[:], lhsT=wq_r[:, dt, ts(do, P)],
                                 rhs=y[:, dt, ds(HALF, CHUNK)],
                                 start=(dt == 0), stop=(dt == DT - 1))
            nc.scalar.activation(qr[:, do, :], pq[:], AF.Identity,
                                 bias=bqt[:, l, do:do + 1], scale=1.0)

        # --- K/V projections, own tokens
        kr = lp.tile([P, DT, W], MDT, tag="kr")
        for do in range(DT):
            pk = ps_a.tile([P, CHUNK], F32, tag="ps_a")
            for dt in range(DT):
                nc.tensor.matmul(out=pk[:], lhsT=wk_r[:, dt, ts(do, P)],
                                 rhs=y[:, dt, ds(HALF, CHUNK)],
                                 start=(dt == 0), stop=(dt == DT - 1))
            nc.scalar.activation(kr[:, do, ds(HALF, CHUNK)], pk[:], AF.Identity,
                                 bias=bkt[:, l, do:do + 1], scale=1.0)

        vt = [lp.tile([P, H * (DH + 1)], MDT, tag=f"vt{t}", name=f"vt{t}") for t in range(W // P)]

        def emit_v(t, ysrc):
            pv = ps_a.tile([P, D], F32, tag="ps_a")
            for dt in range(DT):
                nc.tensor.matmul(out=pv[:], lhsT=ysrc(dt, t), rhs=wv_r[:, dt, :],
                                 start=(dt == 0), stop=(dt == DT - 1))
            vtv = vt[t][:].rearrange("p (h c) -> p h c", c=DH + 1)
            nc.vector.tensor_add(
                out=vtv[:, :, 0:DH],
                in0=pv[:].rearrange("p (h c) -> p h c", c=DH),
                in1=bvt[:, l, :].rearrange("p (h c) -> p h c", c=DH))
            nc.vector.tensor_copy(out=vtv[:, :, DH:DH + 1], in_=ones[:, 0:H])

        for t in range(2, W // P):
            emit_v(t, lambda dt, t: y[:, dt, ts(t, P)])

        # --- sliding-window attention
        attr = lp.tile([P, DT, CHUNK], MDT, tag="attr")
        ej_keep = {}

        def emit_ej(h, kt, keep):
            _, q0, w = KTW[kt]
            r0, dto = (h % 2) * DH, h // 2
            pscore = ps_b.tile([P, 512], F32, tag="ps_b")
            nc.tensor.matmul(out=pscore[:, 0:w],
                             lhsT=kr[ds(r0, DH), dto, ts(kt, P)],
                             rhs=qr[ds(r0, DH), dto, ds(q0, w)],
                             start=True, stop=True)
            ej = lp3.tile([P, 512], MDT, tag="ej_keep" if keep else "ej_tmp",
                          bufs=2 * H if keep else 8, name="ej")
            nc.scalar.activation(ej[:, 0:w], pscore[:, 0:w], AF.Exp,
                                 bias=negb[:], scale=SCALE)
            # mask multiply on gpsimd (SBUF-only operands) — keeps DVE free
            # for the LN/normalization chains
            nc.gpsimd.tensor_mul(out=ej[:, 0:w], in0=ej[:, 0:w], in1=maskt[:, kt, 0:w])
            return ej

        def emit_qblock(h, qB, ejs):
            r0, dto = (h % 2) * DH, h // 2
            pa = ps_c.tile([DH + 1, 256], F32, tag="ps_c")
            for i, kt in enumerate(range(qB * 2, qB * 2 + 4)):
                c0 = qB * 256 - KTW[kt][1]
                nc.tensor.matmul(out=pa[:], lhsT=vt[kt][:, ds(h * (DH + 1), DH + 1)],
                                 rhs=ejs[kt][:, ds(c0, 256)],
                                 start=(i == 0), stop=(i == 3))
            # reciprocal_approx_* misreads PSUM at base_partition!=0 — stage
            # the denominator row to a partition-0 SBUF tile first.
            srow = lp3.tile([1, 256], F32, tag="srow", bufs=4)
            nc.vector.tensor_copy(out=srow[:], in_=pa[DH:DH + 1, :])
            rr = lp3.tile([1, 256], F32, tag="rr", bufs=4)
            nc.vector.reciprocal_approx_fast(out=rr[:], in_=srow[:])
            pbc = ps_a.tile([DH, 256], F32, tag="ps_a")
            nc.tensor.matmul(out=pbc[:], lhsT=ones_f[0:1, 0:DH], rhs=rr[:],
                             start=True, stop=True)
            dst = attr[ds(r0, DH), dto, ds(qB * 256, 256)]
            bcs = lp3.tile([DH, 256], MDT, tag="bcs", bufs=4)
            nc.vector.tensor_copy(out=bcs[:], in_=pbc[:])
            nc.vector.tensor_mul(out=dst, in0=pa[0:DH, :], in1=bcs[:])

        # --- helpers for token-half processing (c0 = 0 or 256)
        def emit_oproj_half(c0):
            for do in range(DT):
                po = ps_a.tile([P, HALF], F32, tag="ps_a")
                for dt in range(DT):
                    nc.tensor.matmul(out=po[:], lhsT=wo_r[:, dt, ts(do, P)],
                                     rhs=attr[:, dt, ds(c0, HALF)],
                                     start=(dt == 0), stop=(dt == DT - 1))
                nc.vector.tensor_add(out=x[:, do, ds(c0, HALF)],
                                     in0=x[:, do, ds(c0, HALF)], in1=po[:])

        def emit_mlp_half(c0):
            emit_ln(srcs=[(lambda dt: x[:, dt, ds(c0, HALF)], c0, HALF)], y=y2)
            pb = [ps_b.tile([P, HALF], F32, tag="ps_b", name=f"pb{i}") for i in range(DT)]

            def emit_mlp2(m, hm):
                for do in range(DT):
                    nc.tensor.matmul(out=pb[do][:], lhsT=w2r[:, m, ts(do, P)],
                                     rhs=hm[:], start=(m == 0), stop=(m == MT - 1))

            hist = []
            for m in range(MT):
                p1 = ps_a.tile([P, HALF], F32, tag="ps_a")
                for dt in range(DT):
                    nc.tensor.matmul(out=p1[:], lhsT=w1r[:, dt, ts(m, P)],
                                     rhs=y2[:, dt, ds(c0, HALF)],
                                     start=(dt == 0), stop=(dt == DT - 1))
                hm = lp3.tile([P, HALF], MDT, tag="hm", bufs=4)
                nc.scalar.activation(hm[:], p1[:], AF.Gelu_apprx_tanh,
                                     bias=b1t[:, l, m:m + 1], scale=1.0)
                hist.append((m, hm))
                if len(hist) > 2:
                    emit_mlp2(*hist.pop(0))
            for mm_, hh_ in hist:
                emit_mlp2(mm_, hh_)
            for do in range(DT):
                nc.vector.scalar_tensor_tensor(
                    out=x[:, do, ds(c0, HALF)], in0=pb[do][:],
                    scalar=b2t[:, l, do:do + 1], in1=x[:, do, ds(c0, HALF)],
                    op0=OP.add, op1=OP.add)

        y2 = lp.tile([P, DT, CHUNK], MDT, tag="y2")

        # import the halo gathered during the previous layer (placed before
        # this layer's collective in the gpsimd queue; waits only on its data)
        if prev_agout is not None:
            for dt in range(DT):
                nc.gpsimd.indirect_dma_start(
                    out=xh[:, dt, :], out_offset=None, in_=prev_agout[:],
                    in_offset=bass.IndirectOffsetOnAxis(ap=hoffs[:, dt:dt + 1], axis=0))

        # phase 1: own-key work for all heads (kt 2..5, qB=1), emitted in
        # waves of 4 heads: all score/exp chains first, then the AV blocks,
        # so the PE sees dense back-to-back matmuls.
        for wave in (range(0, 4), range(4, 8)):
            ejsw = {}
            for h in wave:
                ejsw[h] = {kt: emit_ej(h, kt, kt in (2, 3)) for kt in (2, 3, 4, 5)}
                ej_keep[h] = {kt: ejsw[h][kt] for kt in (2, 3)}
            for h in wave:
                emit_qblock(h, 1, ejsw[h])

        # second token-half completes through its residual, then exports
        emit_oproj_half(HALF)
        emit_mlp_half(HALF)
        if l < NL - 1:
            agin = drp.tile([D, HALF], F32R, tag=f"agin{l}")
            agout = drp.tile([len(GROUPS[0]) * D, HALF], F32R, tag=f"agout{l}")
            nc.sync.dma_start(out=agin[:].rearrange("(t p) m -> p t m", p=P),
                              in_=x[:, :, ds(HALF, HALF)])
            nc.gpsimd.collective_compute(
                "AllGather", OP.bypass, replica_groups=GROUPS,
                ins=[agin.opt()], outs=[agout.opt()])
            prev_agout = agout

        # halo: LN1 on xh, K/V halo columns (consumes prev layer's AllGather)
        emit_ln(srcs=[(lambda dt: xh[:, dt, :], 0, HALF)], y=y)
        for do in range(DT):
            pk = ps_a.tile([P, HALF], F32, tag="ps_a")
            for dt in range(DT):
                nc.tensor.matmul(out=pk[:], lhsT=wk_r[:, dt, ts(do, P)],
                                 rhs=y[:, dt, ds(0, HALF)],
                                 start=(dt == 0), stop=(dt == DT - 1))
            nc.scalar.activation(kr[:, do, ds(0, HALF)], pk[:], AF.Identity,
                                 bias=bkt[:, l, do:do + 1], scale=1.0)
        for t in range(2):
            emit_v(t, lambda dt, t: y[:, dt, ts(t, P)])

        # phase 2: halo-key work (kt 0,1 + kept kt 2,3; qB=0)
        for wave in (range(0, 4), range(4, 8)):
            ejsw = {}
            for h in wave:
                ejsw[h] = dict(ej_keep[h])
                for kt in (0, 1):
                    ejsw[h][kt] = emit_ej(h, kt, False)
            for h in wave:
                emit_qblock(h, 0, ejsw[h])

        if l == 0 and "ydump" in io:
            for nm_t, src_t in (("ydump", y), ("qdump", qr), ("kdump", kr), ("adump", attr)):
                nc.sync.dma_start(out=io[nm_t].ap().rearrange("(t p) m -> p t m", p=P),
                                  in_=src_t[:])

        # prefetch next layer's attention weights (double-buffered pool)
        if l + 1 < _knl:
            wcur = load_qkvo(l + 1)

        # first token-half completes
        emit_oproj_half(0)
        emit_mlp_half(0)

        # prefetch next layer's MLP weights (single buffer: reallocates after use)
        if l + 1 < _knl:
            mcur = load_mlp(l + 1)
        dump_x(l + 1)

    # ------------------------------------------------ final LN + logits
    # Token-sharded final: each core computes the FULL vocab for its own 512
    # tokens from the local yf — no collective, no cross-core skew exposure.
    # w_out (32MB, shared across cores) streams through a small rotating pool;
    # the first few tiles load during the last layer.
    emit_ln(srcs=[(lambda dt: x[:, dt, :], 0, CHUNK)], y=yf)

    lp3.release()
    lp.release()

    FBLK = 1024  # vocab columns per streamed weight tile (1KB DMA segments)
    with tc.tile_pool(name="ftrans", bufs=3) as ftp, \
         tc.tile_pool(name="fout", bufs=6) as fop:
        vb0 = 0
        while vb0 < V:
            vbw = min(FBLK, V - vb0)
            fwr = ftp.tile([P, DT, FBLK], MDT, tag="fwr")
            nc.sync.dma_start(out=fwr[:, :, 0:vbw],
                              in_=io["w_out_sl"].ap()[:, ds(vb0, vbw)]
                              .rearrange("(t p) m -> p t m", p=P))
            for vi in range(vbw // P):
                v_i = vb0 // P + vi
                pf = ps_a.tile([P, CHUNK], F32, tag="ps_a")
                for dt in range(DT):
                    nc.tensor.matmul(out=pf[:], lhsT=fwr[:, dt, ts(vi, P)],
                                     rhs=yf[:, dt, :], start=(dt == 0), stop=(dt == DT - 1))
                ot = fop.tile([P, CHUNK], F16, tag="fot")
                if v_i % 2 == 0:
                    nc.scalar.activation(ot[:], pf[:], AF.Identity,
                                         bias=bot[:, v_i:v_i + 1], scale=1.0)
                else:
                    nc.vector.tensor_scalar_add(out=ot[:], in0=pf[:],
                                                scalar1=bot[:, v_i:v_i + 1])
                nc.sync.dma_start(out=io["out"].ap()[ts(v_i, P), :], in_=ot[:])
            vb0 += vbw

    drp.release()
    ps_c.release()
    ps_b.release()
    ps_a.release()
    wmlp.release()
    wqk.release()
    xpool.release()
    cpool.release()


# ================================================================ host side
def _pe_table():
    pos = np.arange(S, dtype=np.float32)[:, None]
    div = np.exp(np.arange(0, D, 2, dtype=np.float32) * -(np.log(10000.0) / D))
    pe = np.zeros((S, D), dtype=np.float32)
    pe[:, 0::2] = np.sin(pos * div)
    pe[:, 1::2] = np.cos(pos * div)
    return pe


def _in_maps(inputs):
    inp = np.asarray(inputs["inputs"]).astype(np.int32)
    ids = np.pad(inp, ((0, 0), (1, 0)))[:, :-1].astype(np.int32)
    pe = _pe_table()

    f32 = lambda k: np.asarray(inputs[k], dtype=np.float32)
    ln1_s, ln1_b = f32("ln1_s"), f32("ln1_b")
    ln2_s, ln2_b = f32("ln2_s"), f32("ln2_b")
    lnf_s, lnf_b = f32("lnf_s").reshape(D), f32("lnf_b").reshape(D)
    wq, wk, wv, wo = f32("wq"), f32("wk"), f32("wv"), f32("wo")
    w1, w2 = f32("w1"), f32("w2")
    b1, b2 = f32("b1"), f32("b2")
    wout, bout = f32("w_out"), f32("b_out")

    # fold LN affine into the downstream projections
    wq_f = wq * ln1_s[:, :, None]
    wk_f = wk * ln1_s[:, :, None]
    wv_f = wv * ln1_s[:, :, None]
    w1_f = w1 * ln2_s[:, :, None]
    bq = np.einsum("ld,ldm->lm", ln1_b, wq)
    bk = np.einsum("ld,ldm->lm", ln1_b, wk)
    bv = np.einsum("ld,ldm->lm", ln1_b, wv)
    b1_f = b1 + np.einsum("ld,ldm->lm", ln2_b, w1)
    wout_f = wout * lnf_s[:, None]
    bout_f = bout + lnf_b @ wout

    shared = {
        "embed": np.ascontiguousarray(f32("embed")),
        "b1": b1_f, "b2": b2, "bq": bq, "bk": bk,
        "bv": np.ascontiguousarray(np.broadcast_to(bv[None], (P, NL, D)).astype(np.float16)),
        "wq": wq_f.astype(np.float16), "wk": wk_f.astype(np.float16),
        "wv": wv_f.astype(np.float16), "wo": wo.astype(np.float16),
        "w1": w1_f.astype(np.float16), "w2": w2.astype(np.float16),
    }
    shared["w_out_sl"] = np.ascontiguousarray(wout_f.astype(np.float16))
    shared["b_out_sl"] = np.ascontiguousarray(bout_f.reshape(1, V).astype(np.float32))
    shared = {k: np.ascontiguousarray(v) for k, v in shared.items()}

    maps = []
    for c in range(NCORES):
        b, ch = divmod(c, NCORES // B)
        t0 = ch * CHUNK
        lo = t0 - HALF
        ids768 = np.zeros(W, np.int32)
        pe768 = np.zeros((W, D), np.float32)
        s0 = max(0, lo)
        ids768[s0 - lo:] = ids[b, s0:t0 + CHUNK]
        pe768[s0 - lo:] = pe[s0:t0 + CHUNK]
        # per-key-tile masks: [6, 128, 512] f16
        m = np.zeros((6, P, 512), np.float16)
        for kt, q0, w in KTW:
            uk = kt * P + np.arange(P)[:, None]
            q = q0 + np.arange(w)[None, :]
            dqk = (HALF + q) - uk
            ok = (dqk >= 0) & (dqk <= HALF)
            if ch == 0:
                ok = ok & ((lo + uk) >= 0)
            m[kt, :, :w] = ok.astype(np.float16)
        src = ch - 1 if ch > 0 else 0
        hoffs = (src * D + np.arange(DT)[None, :] * P
                 + np.arange(P)[:, None]).astype(np.int32)
        mp = dict(shared)
        mp.update(
            idx_in=np.ascontiguousarray(ids768.reshape(W // P, P).T),
            pe_dm=np.ascontiguousarray(pe768.T),
            masks=m, halo_offs=hoffs)
        maps.append(mp)
    return maps


def _assemble(res):
    full = np.empty((NTOK, V), np.float32)
    for c in range(NCORES):
        full[c * CHUNK:(c + 1) * CHUNK, :] = \
            np.asarray(res[c]["logits_vm"], dtype=np.float32).T
    return full.reshape(B, S, V)


def kernel(**inputs):
    nc = _CACHE.get("nc")
    if nc is None:
        nc = _build()
        _CACHE["nc"] = nc
    maps = _in_maps(inputs)
    res = run_bass_kernel_spmd(nc, maps, list(range(NCORES))).results
    return _assemble(res)


# revision 40
# speedup vs baseline: 1.0233x; 1.0233x over previous
"""Longformer decoder (4 layers, sliding-window causal attention) on 8 trn2 cores.

Sharding: 4096 tokens (B=2 x S=2048) split into 8 contiguous chunks of 512
(core = b*4 + chunk). Activations are kept d-major ([dim, token], dim on
partitions) so every matmul is weights-stationary with no transposes.

v2 structure (vs v1 baseline):
- LN affine (scale/bias) folded into the projection weights host-side; the
  kernel's LN emits the unaffined z=(x-mu)*rstd, with biases re-applied via
  scalar-engine Identity copies (per-partition bias) where needed.
- LN statistics matmuls run on f32r bitcasts of the residual directly (no
  gpsimd f16 staging copies).
- reciprocal_approx_fast for all softmax/LN reciprocals.
- Attention restructured per-head into kt (key-tile) granularity with f16
  masks; per layer, all heads' own-key work (qB=1) runs first so the x-halo
  AllGather from the previous layer is consumed ~40us into the layer.
- Weights double-buffered and prefetched one layer ahead; w_out cached in
  SBUF during the last layer.
- Final logits loop runs the core's own 512-token block first (directly off
  the local yf) while the 8-way AllGather is in flight; remaining blocks are
  imported with per-core indirect DMAs. Output token-blocks are rotated
  per-core ((c+j)%8) so the SPMD program stays identical; host unrotates.
- Logits written f16 (tolerance 2e-2; f16 quantization ~5e-4).
"""
import os
import sys

import numpy as np

for _p in ("/opt/trn_rl_repo", "/root/.axon_site/_ro/trn_rl_repo"):
    if os.path.isdir(_p) and _p not in sys.path:
        sys.path.insert(0, _p)

import concourse.bass as bass
import concourse.mybir as mybir
import concourse.tile as tile
from concourse import bacc
from concourse.bass import ts, ds
from concourse.bass_utils import run_bass_kernel_spmd
from concourse.masks import make_identity

F32 = mybir.dt.float32
F32R = mybir.dt.float32r
F16 = mybir.dt.float16
I32 = mybir.dt.int32
MDT = F16 if os.environ.get("KMMDT", "f16") == "f16" else F32R
AF = mybir.ActivationFunctionType
OP = mybir.AluOpType

B, S, V, D, H, NL, MLPD = 2, 2048, 32000, 512, 8, 4, 2048
DH = D // H            # 64
HALF = 256             # attention half-window (WIN // 2)
P = 128
NCORES = 8
CHUNK = 512            # own tokens per core
W = CHUNK + HALF       # 768 = halo + own
DT = D // P            # 4 d-tiles
MT = MLPD // P         # 16 mlp tiles
VSH = 4096             # padded vocab shard (actual 4000)
NTOK = B * S           # 4096
VV = V // NCORES       # 4000 valid vocab per core
GROUPS = [[0, 1, 2, 3], [4, 5, 6, 7]]
EXP_SHIFT = 2.0
SCALE = float(1.0 / np.sqrt(DH))
# key-tile table: (kt, q0, width) — q columns [q0, q0+width) see key tile kt
KTW = [(0, 0, 256), (1, 0, 256), (2, 0, 512), (3, 0, 512), (4, 256, 256), (5, 256, 256)]

_CACHE = {}


# ================================================================ builder
def _build():
    nc = bacc.Bacc("TRN2", target_bir_lowering=False, debug=False,
                   num_devices=NCORES)

    ein = lambda n, sh, dt=F32: nc.dram_tensor(n, sh, dt, kind="ExternalInput")
    io = dict(
        wq=ein("wq", [NL, D, D], MDT), wk=ein("wk", [NL, D, D], MDT),
        wv=ein("wv", [NL, D, D], MDT), wo=ein("wo", [NL, D, D], MDT),
        w1=ein("w1", [NL, D, MLPD], MDT), w2=ein("w2", [NL, MLPD, D], MDT),
        b1=ein("b1", [NL, MLPD]), b2=ein("b2", [NL, D]),
        bq=ein("bq", [NL, D]), bk=ein("bk", [NL, D]), bv=ein("bv", [P, NL, D], F16),
        w_out_sl=ein("w_out_sl", [D, V], MDT), b_out_sl=ein("b_out_sl", [1, V]),
        embed=ein("embed", [V, D]),
        idx_in=ein("idx_in", [P, W // P], I32),
        pe_dm=ein("pe_dm", [D, W]),
        masks=ein("masks", [6, P, 512], F16),
        halo_offs=ein("halo_offs", [P, DT], I32),
        out=nc.dram_tensor("logits_vm", [V, CHUNK], F16, kind="ExternalOutput"),
    )
    if os.environ.get("KDEBUG") == "1":
        io["xdump"] = nc.dram_tensor("xdump", [NL + 1, D, CHUNK], F32, kind="ExternalOutput")
        io["ydump"] = nc.dram_tensor("ydump", [D, W], MDT, kind="ExternalOutput")
        io["qdump"] = nc.dram_tensor("qdump", [D, CHUNK], MDT, kind="ExternalOutput")
        io["kdump"] = nc.dram_tensor("kdump", [D, W], MDT, kind="ExternalOutput")
        io["adump"] = nc.dram_tensor("adump", [D, CHUNK], MDT, kind="ExternalOutput")

    with tile.TileContext(nc) as tc, nc.allow_low_precision(reason="f32r rounding"):
        _emit(nc, tc, io)
    nc.compile()
    return nc


def _emit(nc, tc, io):
    cpool = tc.alloc_tile_pool(name="const", bufs=1)
    xpool = tc.alloc_tile_pool(name="xres", bufs=1)
    wqk = tc.alloc_tile_pool(name="wqkvo", bufs=2)
    wmlp = tc.alloc_tile_pool(name="wmlp", bufs=1)
    ps_a = tc.alloc_tile_pool(name="ps_a", bufs=2, space="PSUM")
    ps_b = tc.alloc_tile_pool(name="ps_b", bufs=4, space="PSUM")
    ps_c = tc.alloc_tile_pool(name="ps_c", bufs=2, space="PSUM")
    drp = tc.alloc_tile_pool(name="drbounce", bufs=1, space="DRAM")

    # ------------------------------------------------ constants
    ones_f = cpool.tile([P, P], F32, tag="ones_f")
    nc.vector.memset(ones_f[:], 1.0)
    ones = cpool.tile([P, P], MDT, tag="ones")
    nc.vector.tensor_copy(out=ones[:], in_=ones_f[:])
    ones_r = cpool.tile([P, P], F32R, tag="ones_r")
    nc.vector.tensor_copy(out=ones_r[:], in_=ones_f[:])
    negb = cpool.tile([P, 1], F32, tag="negb")
    nc.vector.memset(negb[:], EXP_SHIFT)
    epsb = cpool.tile([P, 1], F32, tag="epsb")
    nc.vector.memset(epsb[:], 1e-6)
    b1t = cpool.tile([P, NL, MT], F32, tag="b1t")
    b2t = cpool.tile([P, NL, DT], F32, tag="b2t")
    bqt = cpool.tile([P, NL, DT], F32, tag="bqt")
    bkt = cpool.tile([P, NL, DT], F32, tag="bkt")
    bvt = cpool.tile([P, NL, D], MDT, tag="bvt")
    bot = cpool.tile([P, V // P], F32, tag="bot")
    maskt = cpool.tile([P, 6, 512], F16, tag="maskt")
    hoffs = cpool.tile([P, DT], I32, tag="hoffs")

    # residual stream (own 512 tokens, d-major) + per-layer halo + final LN out.
    # F32R (same bits as f32) so the LN stat matmuls can consume x directly.
    x = xpool.tile([P, DT, CHUNK], F32R, tag="x")
    xh = xpool.tile([P, DT, HALF], F32R, tag="xh")
    yf = xpool.tile([P, DT, CHUNK], MDT, tag="yf")

    # ------------------------------------------------ weight loads
    def load_qkvo(l):
        w = {}
        for nm in ("wq", "wk", "wv", "wo"):
            t = wqk.tile([P, DT, D], MDT, tag=nm, name=nm)
            nc.sync.dma_start(out=t[:], in_=io[nm].ap()[l].rearrange("(t p) m -> p t m", p=P))
            w[nm] = t
        return w

    def load_mlp(l):
        w1r = wmlp.tile([P, DT, MLPD], MDT, tag="w1r")
        nc.sync.dma_start(out=w1r[:], in_=io["w1"].ap()[l].rearrange("(t p) m -> p t m", p=P))
        w2r = wmlp.tile([P, MT, D], MDT, tag="w2r")
        nc.sync.dma_start(out=w2r[:], in_=io["w2"].ap()[l].rearrange("(t p) m -> p t m", p=P))
        return w1r, w2r

    # ------------------------------------------------ embedding (queued FIRST
    # so the gather/transpose pipeline runs while weights stream in behind it)
    with tc.tile_pool(name="embed", bufs=1) as epool:
        ident = epool.tile([P, P], F32, tag="ident")
        make_identity(nc, ident[:])
        idxt = epool.tile([P, W // P], I32, tag="idxt")
        nc.sync.dma_start(out=idxt[:], in_=io["idx_in"].ap())
        pe = epool.tile([P, DT, W], F32, tag="pe")
        nc.sync.dma_start(out=pe[:], in_=io["pe_dm"].ap().rearrange("(t p) m -> p t m", p=P))
        nc.sync.dma_start(out=b1t[:], in_=io["b1"].ap().rearrange("l (m p) -> p l m", p=P))
        nc.sync.dma_start(out=b2t[:], in_=io["b2"].ap().rearrange("l (t p) -> p l t", p=P))
        nc.sync.dma_start(out=bqt[:], in_=io["bq"].ap().rearrange("l (t p) -> p l t", p=P))
        nc.sync.dma_start(out=bkt[:], in_=io["bk"].ap().rearrange("l (t p) -> p l t", p=P))
        nc.sync.dma_start(out=bvt[:], in_=io["bv"].ap())
        nc.sync.dma_start(out=bot[:], in_=io["b_out_sl"].ap().rearrange("o (t p) -> p (o t)", p=P))
        nc.sync.dma_start(out=maskt[:], in_=io["masks"].ap().rearrange("k p q -> p k q"))
        nc.sync.dma_start(out=hoffs[:], in_=io["halo_offs"].ap())
        wcur = load_qkvo(0)
        mcur = load_mlp(0)
        with tc.tile_pool(name="gath", bufs=2) as gpool:
            for g in range(W // P):
                gt = gpool.tile([P, D], F32, tag="gt")
                nc.gpsimd.indirect_dma_start(
                    out=gt[:], out_offset=None, in_=io["embed"].ap(),
                    in_offset=bass.IndirectOffsetOnAxis(ap=idxt[:, g:g + 1], axis=0),
                )
                for dt in range(DT):
                    pt = ps_a.tile([P, P], F32, tag="ps_a")
                    nc.tensor.transpose(pt[:], gt[:, ts(dt, P)], ident[:])
                    dst = xh[:, dt, ts(g, P)] if g < 2 else x[:, dt, ts(g - 2, P)]
                    nc.vector.tensor_add(out=dst, in0=pt[:], in1=pe[:, dt, ts(g, P)])

    def dump_x(slot):
        if "xdump" in io:
            nc.sync.dma_start(out=io["xdump"].ap()[slot].rearrange("(t p) m -> p t m", p=P),
                              in_=x[:, :, :].bitcast(F32))
    dump_x(0)

    # ------------------------------------------------ layer pools
    lp = tc.alloc_tile_pool(name="layers", bufs=1)
    lp3 = tc.alloc_tile_pool(name="ltrans3", bufs=3)

    def emit_ln(srcs, y):
        """LN over d (partition axis); emits z=(x-mu)*rstd (no affine).
        srcs: list of (fn(dt)->AP[128,width] F32, y_col0, width)."""
        srcs2 = []
        for fn, col0, width in srcs:
            o = 0
            while o < width:
                w = min(512, width - o)
                srcs2.append((lambda dt, fn=fn, o=o, w=w: fn(dt)[:, ds(o, w)], col0 + o, w))
                o += w
        for fn, col0, w in srcs2:
            sx = ps_c.tile([1, 512], F32, tag="ps_c")
            sxx = ps_c.tile([1, 512], F32, tag="ps_c")
            for dt in range(DT):
                xsq = lp3.tile([P, 512], MDT, tag="ln_xsq", bufs=2)
                nc.vector.tensor_mul(out=xsq[:, :w], in0=fn(dt), in1=fn(dt))
                nc.tensor.matmul(out=sx[:, :w], lhsT=ones_r[:, 0:1], rhs=fn(dt),
                                 start=(dt == 0), stop=(dt == DT - 1))
                nc.tensor.matmul(out=sxx[:, :w], lhsT=ones[:, 0:1], rhs=xsq[:, :w],
                                 start=(dt == 0), stop=(dt == DT - 1))
            # murs: [mu | rstd] f16, built by the scalar engine (short chain):
            # mu = sx/D; mu2/var on DVE; rstd = rsqrt(var+eps) directly.
            murs = lp3.tile([1, 1024], MDT, tag="ln_murs", bufs=2)
            mu2 = lp3.tile([1, 512], F32, tag="ln_mu2", bufs=2)
            var = lp3.tile([1, 512], F32, tag="ln_var", bufs=2)
            nc.scalar.activation(murs[:, 0:w], sx[:, :w], AF.Identity, bias=0.0,
                                 scale=1.0 / D)
            nc.vector.tensor_mul(out=mu2[:, :w], in0=murs[:, 0:w], in1=murs[:, 0:w])
            nc.vector.scalar_tensor_tensor(
                out=var[:, :w], in0=sxx[:, :w], scalar=1.0 / D,
                in1=mu2[:, :w], op0=OP.mult, op1=OP.subtract)
            nc.scalar.activation(murs[:, 512:512 + w], var[:, :w],
                                 AF.Abs_reciprocal_sqrt, bias=epsb[0:1, :], scale=1.0)
            if w == 256:
                pmr = ps_a.tile([P, 512], F32, tag="ps_a")
                nc.tensor.matmul(out=pmr[:, 0:256], lhsT=ones[0:1, :],
                                 rhs=murs[:, 0:256], start=True, stop=True)
                nc.tensor.matmul(out=pmr[:, 256:512], lhsT=ones[0:1, :],
                                 rhs=murs[:, 512:768], start=True, stop=True)
                pmu, prs = pmr[:, 0:256], pmr[:, 256:512]
            else:
                pmu_t = ps_a.tile([P, 512], F32, tag="ps_a")
                nc.tensor.matmul(out=pmu_t[:], lhsT=ones[0:1, :], rhs=murs[:, 0:512],
                                 start=True, stop=True)
                prs_t = ps_a.tile([P, 512], F32, tag="ps_a")
                nc.tensor.matmul(out=prs_t[:], lhsT=ones[0:1, :], rhs=murs[:, 512:1024],
                                 start=True, stop=True)
                pmu, prs = pmu_t[:], prs_t[:]
            for dt in range(DT):
                scr = lp3.tile([P, 512], MDT, tag="ln_scr", bufs=2)
                nc.vector.tensor_sub(out=scr[:, :w], in0=fn(dt), in1=pmu)
                nc.vector.tensor_mul(out=y[:, dt, ds(col0, w)], in0=scr[:, :w], in1=prs)

    # ------------------------------------------------ transformer layers
    # Each layer is split into token-halves: the second half (tokens 256-511)
    # finishes first — through attention qB=1, O-proj, MLP, residual — and its
    # x is exported + AllGathered mid-layer. The gather-dependent first half
    # (halo LN, K/V halo, qB=0) of the NEXT layer then trails the export by a
    # full half-layer of independent work, hiding collective latency + skew.
    _knl = int(os.environ.get("KNL", NL))
    prev_agout = None

    def emit_halo(hl, y_t, kr_t, vt01, wk_t, wv_t):
        """LN + K/V projections for layer hl's halo tokens (reads xh).
        Hoisted into the previous layer's tail: its DVE chain overlaps the
        MLP matmul stream, and its matmuls fill the next layer's LN1 chain."""
        emit_ln(srcs=[(lambda dt: xh[:, dt, :], 0, HALF)], y=y_t)
        for do in range(DT):
            pk = ps_a.tile([P, HALF], F32, tag="ps_a")
            for dt in range(DT):
                nc.tensor.matmul(out=pk[:], lhsT=wk_t[:, dt, ts(do, P)],
                                 rhs=y_t[:, dt, ds(0, HALF)],
                                 start=(dt == 0), stop=(dt == DT - 1))
            nc.scalar.activation(kr_t[:, do, ds(0, HALF)], pk[:], AF.Identity,
                                 bias=bkt[:, hl, do:do + 1], scale=1.0)
        for t in range(2):
            pv = ps_a.tile([P, D], F32, tag="ps_a")
            for dt in range(DT):
                nc.tensor.matmul(out=pv[:], lhsT=y_t[:, dt, ts(t, P)], rhs=wv_t[:, dt, :],
                                 start=(dt == 0), stop=(dt == DT - 1))
            vtv = vt01[t][:].rearrange("p (h c) -> p h c", c=DH + 1)
            nc.vector.tensor_add(
                out=vtv[:, :, 0:DH],
                in0=pv[:].rearrange("p (h c) -> p h c", c=DH),
                in1=bvt[:, hl, :].rearrange("p (h c) -> p h c", c=DH))
            nc.vector.tensor_copy(out=vtv[:, :, DH:DH + 1], in_=ones[:, 0:H])

    def alloc_head_tiles():
        y_t = lp.tile([P, DT, W], MDT, tag="y", bufs=2, name="y")
        kr_t = lp.tile([P, DT, W], MDT, tag="kr", bufs=2, name="kr")
        vt01 = [lp.tile([P, H * (DH + 1)], MDT, tag=f"vt{t}", bufs=2, name=f"vt{t}")
                for t in range(2)]
        return y_t, kr_t, vt01

    # layer 0's halo comes straight from the embedding
    pend = alloc_head_tiles()
    emit_halo(0, pend[0], pend[1], pend[2], wcur["wk"], wcur["wv"])

    for l in range(_knl):
        wq_r, wk_r, wv_r, wo_r = wcur["wq"], wcur["wk"], wcur["wv"], wcur["wo"]
        w1r, w2r = mcur
        y, kr, vt01 = pend
        vt = vt01 + [lp.tile([P, H * (DH + 1)], MDT, tag=f"vtb{t}", name=f"vt{t}")
                     for t in range(2, W // P)]

        # LN1 on own tokens (halo columns were filled last layer)
        emit_ln(srcs=[(lambda dt: x[:, dt, :], HALF, CHUNK)], y=y)

        # --- Q projection (own tokens only)
        qr = lp.tile([P, DT, CHUNK], MDT, tag="qr")
        for do in range(DT):
            pq = ps_a.tile([P, CHUNK], F32, tag="ps_a")
            for dt in range(DT):
                nc.tensor.matmul(out=pq[:], lhsT=wq_r[:, dt, ts(do, P)],
                                 rhs=y[:, dt, ds(HALF, CHUNK)],
                                 start=(dt == 0), stop=(dt == DT - 1))
            nc.scalar.activation(qr[:, do, :], pq[:], AF.Identity,
                                 bias=bqt[:, l, do:do + 1], scale=1.0)

        # --- K/V projections, own tokens
        for do in range(DT):
            pk = ps_a.tile([P, CHUNK], F32, tag="ps_a")
            for dt in range(DT):
                nc.tensor.matmul(out=pk[:], lhsT=wk_r[:, dt, ts(do, P)],
                                 rhs=y[:, dt, ds(HALF, CHUNK)],
                                 start=(dt == 0), stop=(dt == DT - 1))
            nc.scalar.activation(kr[:, do, ds(HALF, CHUNK)], pk[:], AF.Identity,
                                 bias=bkt[:, l, do:do + 1], scale=1.0)

        for t in range(2, W // P):
            pv = ps_a.tile([P, D], F32, tag="ps_a")
            for dt in range(DT):
                nc.tensor.matmul(out=pv[:], lhsT=y[:, dt, ts(t, P)], rhs=wv_r[:, dt, :],
                                 start=(dt == 0), stop=(dt == DT - 1))
            vtv = vt[t][:].rearrange("p (h c) -> p h c", c=DH + 1)
            nc.vector.tensor_add(
                out=vtv[:, :, 0:DH],
                in0=pv[:].rearrange("p (h c) -> p h c", c=DH),
                in1=bvt[:, l, :].rearrange("p (h c) -> p h c", c=DH))
            nc.vector.tensor_copy(out=vtv[:, :, DH:DH + 1], in_=ones[:, 0:H])

        # --- sliding-window attention
        attr = lp.tile([P, DT, CHUNK], MDT, tag="attr")
        ej_keep = {}

        def emit_ej(h, kt, keep):
            _, q0, w = KTW[kt]
            r0, dto = (h % 2) * DH, h // 2
            pscore = ps_b.tile([P, 512], F32, tag="ps_b")
            nc.tensor.matmul(out=pscore[:, 0:w],
                             lhsT=kr[ds(r0, DH), dto, ts(kt, P)],
                             rhs=qr[ds(r0, DH), dto, ds(q0, w)],
                             start=True, stop=True)
            ej = lp3.tile([P, 512], MDT, tag="ej_keep" if keep else "ej_tmp",
                          bufs=2 * H if keep else 8, name="ej")
            nc.scalar.activation(ej[:, 0:w], pscore[:, 0:w], AF.Exp,
                                 bias=negb[:], scale=SCALE)
            nc.vector.tensor_mul(out=ej[:, 0:w], in0=ej[:, 0:w], in1=maskt[:, kt, 0:w])
            return ej

        def emit_qblock(h, qB, ejs):
            r0, dto = (h % 2) * DH, h // 2
            pa = ps_c.tile([DH + 1, 256], F32, tag="ps_c")
            for i, kt in enumerate(range(qB * 2, qB * 2 + 4)):
                c0 = qB * 256 - KTW[kt][1]
                nc.tensor.matmul(out=pa[:], lhsT=vt[kt][:, ds(h * (DH + 1), DH + 1)],
                                 rhs=ejs[kt][:, ds(c0, 256)],
                                 start=(i == 0), stop=(i == 3))
            # reciprocal_approx_* misreads PSUM at base_partition!=0 — stage
            # the denominator row (partition 64) to partition-0 SBUF first
            srow = lp3.tile([1, 256], F32, tag="srow", bufs=4)
            nc.vector.tensor_copy(out=srow[:], in_=pa[DH:DH + 1, :])
            rr = lp3.tile([1, 256], F32, tag="rr", bufs=4)
            nc.vector.reciprocal_approx_fast(out=rr[:], in_=srow[:])
            pbc = ps_a.tile([DH, 256], F32, tag="ps_a")
            nc.tensor.matmul(out=pbc[:], lhsT=ones_f[0:1, 0:DH], rhs=rr[:],
                             start=True, stop=True)
            dst = attr[ds(r0, DH), dto, ds(qB * 256, 256)]
            bcs = lp3.tile([DH, 256], MDT, tag="bcs", bufs=4)
            nc.vector.tensor_copy(out=bcs[:], in_=pbc[:])
            nc.vector.tensor_mul(out=dst, in0=pa[0:DH, :], in1=bcs[:])

        # --- helpers for token-half processing (c0 = 0 or 256)
        def emit_oproj_half(c0):
            for do in range(DT):
                po = ps_a.tile([P, HALF], F32, tag="ps_a")
                for dt in range(DT):
                    nc.tensor.matmul(out=po[:], lhsT=wo_r[:, dt, ts(do, P)],
                                     rhs=attr[:, dt, ds(c0, HALF)],
                                     start=(dt == 0), stop=(dt == DT - 1))
                nc.vector.tensor_add(out=x[:, do, ds(c0, HALF)],
                                     in0=x[:, do, ds(c0, HALF)], in1=po[:])

        def emit_mlp_half(c0):
            emit_ln(srcs=[(lambda dt: x[:, dt, ds(c0, HALF)], c0, HALF)], y=y2)
            pb = [ps_b.tile([P, HALF], F32, tag="ps_b", name=f"pb{i}") for i in range(DT)]

            def emit_mlp2(m, hm):
                for do in range(DT):
                    nc.tensor.matmul(out=pb[do][:], lhsT=w2r[:, m, ts(do, P)],
                                     rhs=hm[:], start=(m == 0), stop=(m == MT - 1))

            hist = []
            for m in range(MT):
                p1 = ps_a.tile([P, HALF], F32, tag="ps_a")
                for dt in range(DT):
                    nc.tensor.matmul(out=p1[:], lhsT=w1r[:, dt, ts(m, P)],
                                     rhs=y2[:, dt, ds(c0, HALF)],
                                     start=(dt == 0), stop=(dt == DT - 1))
                hm = lp3.tile([P, HALF], MDT, tag="hm", bufs=4)
                nc.scalar.activation(hm[:], p1[:], AF.Gelu_apprx_tanh,
                                     bias=b1t[:, l, m:m + 1], scale=1.0)
                hist.append((m, hm))
                if len(hist) > 2:
                    emit_mlp2(*hist.pop(0))
            for mm_, hh_ in hist:
                emit_mlp2(mm_, hh_)
            for do in range(DT):
                nc.vector.scalar_tensor_tensor(
                    out=x[:, do, ds(c0, HALF)], in0=pb[do][:],
                    scalar=b2t[:, l, do:do + 1], in1=x[:, do, ds(c0, HALF)],
                    op0=OP.add, op1=OP.add)

        y2 = lp.tile([P, DT, CHUNK], MDT, tag="y2")

        # phase 1: own-key work for all heads (kt 2..5, qB=1), emitted in
        # waves of 4 heads: all score/exp chains first, then the AV blocks,
        # so the PE sees dense back-to-back matmuls.
        for wave in (range(0, 4), range(4, 8)):
            ejsw = {}
            for h in wave:
                ejsw[h] = {kt: emit_ej(h, kt, kt in (2, 3)) for kt in (2, 3, 4, 5)}
                ej_keep[h] = {kt: ejsw[h][kt] for kt in (2, 3)}
            for h in wave:
                emit_qblock(h, 1, ejsw[h])

        # second token-half completes through its residual, then exports
        emit_oproj_half(HALF)
        emit_mlp_half(HALF)
        if l < NL - 1:
            agin = drp.tile([D, HALF], F32R, tag=f"agin{l}")
            agout = drp.tile([len(GROUPS[0]) * D, HALF], F32R, tag=f"agout{l}")
            nc.sync.dma_start(out=agin[:].rearrange("(t p) m -> p t m", p=P),
                              in_=x[:, :, ds(HALF, HALF)])
            nc.gpsimd.collective_compute(
                "AllGather", OP.bypass, replica_groups=GROUPS,
                ins=[agin.opt()], outs=[agout.opt()])
            prev_agout = agout

        # phase 2: halo-key work (kt 0,1 + kt 2,3 kept from phase 1; qB=0).
        # The halo K/V were computed at the END of the previous layer.
        for wave in (range(0, 4), range(4, 8)):
            ejsw = {}
            for h in wave:
                ejsw[h] = dict(ej_keep[h])
                for kt in (0, 1):
                    ejsw[h][kt] = emit_ej(h, kt, False)
            for h in wave:
                emit_qblock(h, 0, ejsw[h])

        if l == 0 and "ydump" in io:
            for nm_t, src_t in (("ydump", y), ("qdump", qr), ("kdump", kr), ("adump", attr)):
                nc.sync.dma_start(out=io[nm_t].ap().rearrange("(t p) m -> p t m", p=P),
                                  in_=src_t[:])

        # prefetch next layer's attention weights (double-buffered pool)
        if l + 1 < _knl:
            wcur = load_qkvo(l + 1)

        # first token-half completes
        emit_oproj_half(0)
        emit_mlp_half(0)

        if l + 1 < _knl:
            mcur = load_mlp(l + 1)
            # import the halo gathered this layer, then hoist the next layer's
            # halo LN + K/V here (overlaps this layer's MLP tail / next LN1)
            for dt in range(DT):
                nc.gpsimd.indirect_dma_start(
                    out=xh[:, dt, :], out_offset=None, in_=prev_agout[:],
                    in_offset=bass.IndirectOffsetOnAxis(ap=hoffs[:, dt:dt + 1], axis=0))
            pend = alloc_head_tiles()
            emit_halo(l + 1, pend[0], pend[1], pend[2], wcur["wk"], wcur["wv"])
        dump_x(l + 1)

    # ------------------------------------------------ final LN + logits
    # Token-sharded final: each core computes the FULL vocab for its own 512
    # tokens from the local yf — no collective, no cross-core skew exposure.
    # w_out (32MB, shared across cores) streams through a small rotating pool;
    # the first few tiles load during the last layer.
    emit_ln(srcs=[(lambda dt: x[:, dt, :], 0, CHUNK)], y=yf)

    lp3.release()
    lp.release()

    FBLK = 1024  # vocab columns per streamed weight tile (1KB DMA segments)
    with tc.tile_pool(name="ftrans", bufs=3) as ftp, \
         tc.tile_pool(name="fout", bufs=6) as fop:
        vb0 = 0
        while vb0 < V:
            vbw = min(FBLK, V - vb0)
            fwr = ftp.tile([P, DT, FBLK], MDT, tag="fwr")
            nc.sync.dma_start(out=fwr[:, :, 0:vbw],
                              in_=io["w_out_sl"].ap()[:, ds(vb0, vbw)]
                              .rearrange("(t p) m -> p t m", p=P))
            for vi in range(vbw // P):
                v_i = vb0 // P + vi
                pf = ps_a.tile([P, CHUNK], F32, tag="ps_a")
                for dt in range(DT):
                    nc.tensor.matmul(out=pf[:], lhsT=fwr[:, dt, ts(vi, P)],
                                     rhs=yf[:, dt, :], start=(dt == 0), stop=(dt == DT - 1))
                ot = fop.tile([P, CHUNK], F16, tag="fot")
                if v_i % 2 == 0:
                    nc.scalar.activation(ot[:], pf[:], AF.Identity,
                                         bias=bot[:, v_i:v_i + 1], scale=1.0)
                else:
                    nc.vector.tensor_scalar_add(out=ot[:], in0=pf[:],
                                                scalar1=bot[:, v_i:v_i + 1])
                nc.sync.dma_start(out=io["out"].ap()[ts(v_i, P), :], in_=ot[:])
            vb0 += vbw

    drp.release()
    ps_c.release()
    ps_b.release()
    ps_a.release()
    wmlp.release()
    wqk.release()
    xpool.release()
    cpool.release()


# ================================================================ host side
def _pe_table():
    pos = np.arange(S, dtype=np.float32)[:, None]
    div = np.exp(np.arange(0, D, 2, dtype=np.float32) * -(np.log(10000.0) / D))
    pe = np.zeros((S, D), dtype=np.float32)
    pe[:, 0::2] = np.sin(pos * div)
    pe[:, 1::2] = np.cos(pos * div)
    return pe


def _in_maps(inputs):
    inp = np.asarray(inputs["inputs"]).astype(np.int32)
    ids = np.pad(inp, ((0, 0), (1, 0)))[:, :-1].astype(np.int32)
    pe = _pe_table()

    f32 = lambda k: np.asarray(inputs[k], dtype=np.float32)
    ln1_s, ln1_b = f32("ln1_s"), f32("ln1_b")
    ln2_s, ln2_b = f32("ln2_s"), f32("ln2_b")
    lnf_s, lnf_b = f32("lnf_s").reshape(D), f32("lnf_b").reshape(D)
    wq, wk, wv, wo = f32("wq"), f32("wk"), f32("wv"), f32("wo")
    w1, w2 = f32("w1"), f32("w2")
    b1, b2 = f32("b1"), f32("b2")
    wout, bout = f32("w_out"), f32("b_out")

    # fold LN affine into the downstream projections
    wq_f = wq * ln1_s[:, :, None]
    wk_f = wk * ln1_s[:, :, None]
    wv_f = wv * ln1_s[:, :, None]
    w1_f = w1 * ln2_s[:, :, None]
    bq = np.einsum("ld,ldm->lm", ln1_b, wq)
    bk = np.einsum("ld,ldm->lm", ln1_b, wk)
    bv = np.einsum("ld,ldm->lm", ln1_b, wv)
    b1_f = b1 + np.einsum("ld,ldm->lm", ln2_b, w1)
    wout_f = wout * lnf_s[:, None]
    bout_f = bout + lnf_b @ wout

    shared = {
        "embed": np.ascontiguousarray(f32("embed")),
        "b1": b1_f, "b2": b2, "bq": bq, "bk": bk,
        "bv": np.ascontiguousarray(np.broadcast_to(bv[None], (P, NL, D)).astype(np.float16)),
        "wq": wq_f.astype(np.float16), "wk": wk_f.astype(np.float16),
        "wv": wv_f.astype(np.float16), "wo": wo.astype(np.float16),
        "w1": w1_f.astype(np.float16), "w2": w2.astype(np.float16),
    }
    shared["w_out_sl"] = np.ascontiguousarray(wout_f.astype(np.float16))
    shared["b_out_sl"] = np.ascontiguousarray(bout_f.reshape(1, V).astype(np.float32))
    shared = {k: np.ascontiguousarray(v) for k, v in shared.items()}

    maps = []
    for c in range(NCORES):
        b, ch = divmod(c, NCORES // B)
        t0 = ch * CHUNK
        lo = t0 - HALF
        ids768 = np.zeros(W, np.int32)
        pe768 = np.zeros((W, D), np.float32)
        s0 = max(0, lo)
        ids768[s0 - lo:] = ids[b, s0:t0 + CHUNK]
        pe768[s0 - lo:] = pe[s0:t0 + CHUNK]
        # per-key-tile masks: [6, 128, 512] f16
        m = np.zeros((6, P, 512), np.float16)
        for kt, q0, w in KTW:
            uk = kt * P + np.arange(P)[:, None]
            q = q0 + np.arange(w)[None, :]
            dqk = (HALF + q) - uk
            ok = (dqk >= 0) & (dqk <= HALF)
            if ch == 0:
                ok = ok & ((lo + uk) >= 0)
            m[kt, :, :w] = ok.astype(np.float16)
        src = ch - 1 if ch > 0 else 0
        hoffs = (src * D + np.arange(DT)[None, :] * P
                 + np.arange(P)[:, None]).astype(np.int32)
        mp = dict(shared)
        mp.update(
            idx_in=np.ascontiguousarray(ids768.reshape(W // P, P).T),
            pe_dm=np.ascontiguousarray(pe768.T),
            masks=m, halo_offs=hoffs)
        maps.append(mp)
    return maps


def _assemble(res):
    full = np.empty((NTOK, V), np.float32)
    for c in range(NCORES):
        full[c * CHUNK:(c + 1) * CHUNK, :] = \
            np.asarray(res[c]["logits_vm"], dtype=np.float32).T
    return full.reshape(B, S, V)


def kernel(**inputs):
    nc = _CACHE.get("nc")
    if nc is None:
        nc = _build()
        _CACHE["nc"] = nc
    maps = _in_maps(inputs)
    res = run_bass_kernel_spmd(nc, maps, list(range(NCORES))).results
    return _assemble(res)


# revision 41
# speedup vs baseline: 1.1160x; 1.0906x over previous
"""Longformer decoder (4 layers, sliding-window causal attention) on 8 trn2 cores.

Sharding: 4096 tokens (B=2 x S=2048) split into 8 contiguous chunks of 512
(core = b*4 + chunk). Activations are kept d-major ([dim, token], dim on
partitions) so every matmul is weights-stationary with no transposes.

v2 structure (vs v1 baseline):
- LN affine (scale/bias) folded into the projection weights host-side; the
  kernel's LN emits the unaffined z=(x-mu)*rstd, with biases re-applied via
  scalar-engine Identity copies (per-partition bias) where needed.
- LN statistics matmuls run on f32r bitcasts of the residual directly (no
  gpsimd f16 staging copies).
- reciprocal_approx_fast for all softmax/LN reciprocals.
- Attention restructured per-head into kt (key-tile) granularity with f16
  masks; per layer, all heads' own-key work (qB=1) runs first so the x-halo
  AllGather from the previous layer is consumed ~40us into the layer.
- Weights double-buffered and prefetched one layer ahead; w_out cached in
  SBUF during the last layer.
- Final logits loop runs the core's own 512-token block first (directly off
  the local yf) while the 8-way AllGather is in flight; remaining blocks are
  imported with per-core indirect DMAs. Output token-blocks are rotated
  per-core ((c+j)%8) so the SPMD program stays identical; host unrotates.
- Logits written f16 (tolerance 2e-2; f16 quantization ~5e-4).
"""
import os
import sys

import numpy as np

for _p in ("/opt/trn_rl_repo", "/root/.axon_site/_ro/trn_rl_repo"):
    if os.path.isdir(_p) and _p not in sys.path:
        sys.path.insert(0, _p)

import concourse.bass as bass
import concourse.mybir as mybir
import concourse.tile as tile
from concourse import bacc
from concourse.bass import ts, ds
from concourse.bass_utils import run_bass_kernel_spmd
from concourse.masks import make_identity

F32 = mybir.dt.float32
F32R = mybir.dt.float32r
F16 = mybir.dt.float16
I32 = mybir.dt.int32
MDT = F16 if os.environ.get("KMMDT", "f16") == "f16" else F32R
AF = mybir.ActivationFunctionType
OP = mybir.AluOpType

B, S, V, D, H, NL, MLPD = 2, 2048, 32000, 512, 8, 4, 2048
DH = D // H            # 64
HALF = 256             # attention half-window (WIN // 2)
P = 128
NCORES = 8
CHUNK = 512            # own tokens per core
W = CHUNK + HALF       # 768 = halo + own
DT = D // P            # 4 d-tiles
MT = MLPD // P         # 16 mlp tiles
VSH = 4096             # padded vocab shard (actual 4000)
NTOK = B * S           # 4096
VV = V // NCORES       # 4000 valid vocab per core
GROUPS = [[0, 1, 2, 3], [4, 5, 6, 7]]
EXP_SHIFT = 2.0
SCALE = float(1.0 / np.sqrt(DH))
# key-tile table: (kt, q0, width) — q columns [q0, q0+width) see key tile kt
KTW = [(0, 0, 256), (1, 0, 256), (2, 0, 512), (3, 0, 512), (4, 256, 256), (5, 256, 256)]

_CACHE = {}


# ================================================================ builder
def _build():
    nc = bacc.Bacc("TRN2", target_bir_lowering=False, debug=False,
                   num_devices=NCORES)

    ein = lambda n, sh, dt=F32: nc.dram_tensor(n, sh, dt, kind="ExternalInput")
    io = dict(
        wq=ein("wq", [NL, D, D], MDT), wk=ein("wk", [NL, D, D], MDT),
        wv=ein("wv", [NL, D, D], MDT), wo=ein("wo", [NL, D, D], MDT),
        w1=ein("w1", [NL, D, MLPD], MDT), w2=ein("w2", [NL, MLPD, D], MDT),
        b1=ein("b1", [NL, MLPD]), b2=ein("b2", [NL, D]),
        bq=ein("bq", [NL, D]), bk=ein("bk", [NL, D]), bv=ein("bv", [P, NL, D], F16),
        w_out_sl=ein("w_out_sl", [D, V], MDT), b_out_sl=ein("b_out_sl", [1, V]),
        embed=ein("embed", [V, D]),
        idx_in=ein("idx_in", [P, W // P], I32),
        pe_dm=ein("pe_dm", [D, W]),
        masks=ein("masks", [6, P, 512], F16),
        halo_offs=ein("halo_offs", [P, DT], I32),
        out=nc.dram_tensor("logits_vm", [V, CHUNK], F16, kind="ExternalOutput"),
    )
    if os.environ.get("KDEBUG") == "1":
        io["xdump"] = nc.dram_tensor("xdump", [NL + 1, D, CHUNK], F32, kind="ExternalOutput")
        io["ydump"] = nc.dram_tensor("ydump", [D, W], MDT, kind="ExternalOutput")
        io["qdump"] = nc.dram_tensor("qdump", [D, CHUNK], MDT, kind="ExternalOutput")
        io["kdump"] = nc.dram_tensor("kdump", [D, W], MDT, kind="ExternalOutput")
        io["adump"] = nc.dram_tensor("adump", [D, CHUNK], MDT, kind="ExternalOutput")

    with tile.TileContext(nc) as tc, nc.allow_low_precision(reason="f32r rounding"):
        _emit(nc, tc, io)
    nc.compile()
    return nc


def _emit(nc, tc, io):
    cpool = tc.alloc_tile_pool(name="const", bufs=1)
    xpool = tc.alloc_tile_pool(name="xres", bufs=1)
    wqk = tc.alloc_tile_pool(name="wqkvo", bufs=2)
    wmlp = tc.alloc_tile_pool(name="wmlp", bufs=1)
    ps_a = tc.alloc_tile_pool(name="ps_a", bufs=2, space="PSUM")
    ps_b = tc.alloc_tile_pool(name="ps_b", bufs=4, space="PSUM")
    ps_c = tc.alloc_tile_pool(name="ps_c", bufs=2, space="PSUM")
    drp = tc.alloc_tile_pool(name="drbounce", bufs=1, space="DRAM")

    # ------------------------------------------------ constants
    ones_f = cpool.tile([P, P], F32, tag="ones_f")
    nc.vector.memset(ones_f[:], 1.0)
    ones = cpool.tile([P, P], MDT, tag="ones")
    nc.vector.tensor_copy(out=ones[:], in_=ones_f[:])
    ones_r = cpool.tile([P, P], F32R, tag="ones_r")
    nc.vector.tensor_copy(out=ones_r[:], in_=ones_f[:])
    negb = cpool.tile([P, 1], F32, tag="negb")
    nc.vector.memset(negb[:], EXP_SHIFT)
    epsb = cpool.tile([P, 1], F32, tag="epsb")
    nc.vector.memset(epsb[:], 1e-6)
    b1t = cpool.tile([P, NL, MT], F32, tag="b1t")
    b2t = cpool.tile([P, NL, DT], F32, tag="b2t")
    bqt = cpool.tile([P, NL, DT], F32, tag="bqt")
    bkt = cpool.tile([P, NL, DT], F32, tag="bkt")
    bvt = cpool.tile([P, NL, D], MDT, tag="bvt")
    bot = cpool.tile([P, V // P], F32, tag="bot")
    maskt = cpool.tile([P, 6, 512], F16, tag="maskt")
    hoffs = cpool.tile([P, DT], I32, tag="hoffs")

    # residual stream (own 512 tokens, d-major) + per-layer halo + final LN out.
    # F32R (same bits as f32) so the LN stat matmuls can consume x directly.
    x = xpool.tile([P, DT, CHUNK], F32R, tag="x")
    xh = xpool.tile([P, DT, HALF], F32R, tag="xh")
    yf = xpool.tile([P, DT, CHUNK], MDT, tag="yf")

    # ------------------------------------------------ weight loads
    def load_qkvo(l):
        w = {}
        for nm in ("wq", "wk", "wv", "wo"):
            t = wqk.tile([P, DT, D], MDT, tag=nm, name=nm)
            nc.sync.dma_start(out=t[:], in_=io[nm].ap()[l].rearrange("(t p) m -> p t m", p=P))
            w[nm] = t
        return w

    def load_mlp(l):
        w1r = wmlp.tile([P, DT, MLPD], MDT, tag="w1r")
        nc.sync.dma_start(out=w1r[:], in_=io["w1"].ap()[l].rearrange("(t p) m -> p t m", p=P))
        w2r = wmlp.tile([P, MT, D], MDT, tag="w2r")
        nc.sync.dma_start(out=w2r[:], in_=io["w2"].ap()[l].rearrange("(t p) m -> p t m", p=P))
        return w1r, w2r

    # ------------------------------------------------ embedding (queued FIRST
    # so the gather/transpose pipeline runs while weights stream in behind it)
    with tc.tile_pool(name="embed", bufs=1) as epool:
        ident = epool.tile([P, P], F32, tag="ident")
        make_identity(nc, ident[:])
        idxt = epool.tile([P, W // P], I32, tag="idxt")
        nc.sync.dma_start(out=idxt[:], in_=io["idx_in"].ap())
        pe = epool.tile([P, DT, W], F32, tag="pe")
        nc.sync.dma_start(out=pe[:], in_=io["pe_dm"].ap().rearrange("(t p) m -> p t m", p=P))
        nc.sync.dma_start(out=b1t[:], in_=io["b1"].ap().rearrange("l (m p) -> p l m", p=P))
        nc.sync.dma_start(out=b2t[:], in_=io["b2"].ap().rearrange("l (t p) -> p l t", p=P))
        nc.sync.dma_start(out=bqt[:], in_=io["bq"].ap().rearrange("l (t p) -> p l t", p=P))
        nc.sync.dma_start(out=bkt[:], in_=io["bk"].ap().rearrange("l (t p) -> p l t", p=P))
        nc.sync.dma_start(out=bvt[:], in_=io["bv"].ap())
        nc.sync.dma_start(out=bot[:], in_=io["b_out_sl"].ap().rearrange("o (t p) -> p (o t)", p=P))
        nc.sync.dma_start(out=maskt[:], in_=io["masks"].ap().rearrange("k p q -> p k q"))
        nc.sync.dma_start(out=hoffs[:], in_=io["halo_offs"].ap())
        wcur = load_qkvo(0)
        mcur = load_mlp(0)
        with tc.tile_pool(name="gath", bufs=2) as gpool:
            for g in range(W // P):
                gt = gpool.tile([P, D], F32, tag="gt")
                nc.gpsimd.indirect_dma_start(
                    out=gt[:], out_offset=None, in_=io["embed"].ap(),
                    in_offset=bass.IndirectOffsetOnAxis(ap=idxt[:, g:g + 1], axis=0),
                )
                for dt in range(DT):
                    pt = ps_a.tile([P, P], F32, tag="ps_a")
                    nc.tensor.transpose(pt[:], gt[:, ts(dt, P)], ident[:])
                    dst = xh[:, dt, ts(g, P)] if g < 2 else x[:, dt, ts(g - 2, P)]
                    nc.vector.tensor_add(out=dst, in0=pt[:], in1=pe[:, dt, ts(g, P)])

    def dump_x(slot):
        if "xdump" in io:
            nc.sync.dma_start(out=io["xdump"].ap()[slot].rearrange("(t p) m -> p t m", p=P),
                              in_=x[:, :, :].bitcast(F32))
    dump_x(0)

    # ------------------------------------------------ layer pools
    lp = tc.alloc_tile_pool(name="layers", bufs=1)
    lp3 = tc.alloc_tile_pool(name="ltrans3", bufs=3)

    def emit_ln(srcs, y):
        """LN over d (partition axis); emits z=(x-mu)*rstd (no affine).
        srcs: list of (fn(dt)->AP[128,width] F32, y_col0, width)."""
        srcs2 = []
        for fn, col0, width in srcs:
            o = 0
            while o < width:
                w = min(512, width - o)
                srcs2.append((lambda dt, fn=fn, o=o, w=w: fn(dt)[:, ds(o, w)], col0 + o, w))
                o += w
        for fn, col0, w in srcs2:
            sx = ps_c.tile([1, 512], F32, tag="ps_c")
            sxx = ps_c.tile([1, 512], F32, tag="ps_c")
            for dt in range(DT):
                xsq = lp3.tile([P, 512], MDT, tag="ln_xsq", bufs=2)
                nc.vector.tensor_mul(out=xsq[:, :w], in0=fn(dt), in1=fn(dt))
                nc.tensor.matmul(out=sx[:, :w], lhsT=ones_r[:, 0:1], rhs=fn(dt),
                                 start=(dt == 0), stop=(dt == DT - 1))
                nc.tensor.matmul(out=sxx[:, :w], lhsT=ones[:, 0:1], rhs=xsq[:, :w],
                                 start=(dt == 0), stop=(dt == DT - 1))
            # murs: [mu | rstd] f16, built by the scalar engine (short chain):
            # mu = sx/D; mu2/var on DVE; rstd = rsqrt(var+eps) directly.
            murs = lp3.tile([1, 1024], MDT, tag="ln_murs", bufs=2)
            mu2 = lp3.tile([1, 512], F32, tag="ln_mu2", bufs=2)
            var = lp3.tile([1, 512], F32, tag="ln_var", bufs=2)
            nc.scalar.activation(murs[:, 0:w], sx[:, :w], AF.Identity, bias=0.0,
                                 scale=1.0 / D)
            nc.vector.tensor_mul(out=mu2[:, :w], in0=murs[:, 0:w], in1=murs[:, 0:w])
            nc.vector.scalar_tensor_tensor(
                out=var[:, :w], in0=sxx[:, :w], scalar=1.0 / D,
                in1=mu2[:, :w], op0=OP.mult, op1=OP.subtract)
            nc.scalar.activation(murs[:, 512:512 + w], var[:, :w],
                                 AF.Abs_reciprocal_sqrt, bias=epsb[0:1, :], scale=1.0)
            if w == 256:
                pmr = ps_a.tile([P, 512], F32, tag="ps_a")
                nc.tensor.matmul(out=pmr[:, 0:256], lhsT=ones[0:1, :],
                                 rhs=murs[:, 0:256], start=True, stop=True)
                nc.tensor.matmul(out=pmr[:, 256:512], lhsT=ones[0:1, :],
                                 rhs=murs[:, 512:768], start=True, stop=True)
                pmu, prs = pmr[:, 0:256], pmr[:, 256:512]
            else:
                pmu_t = ps_a.tile([P, 512], F32, tag="ps_a")
                nc.tensor.matmul(out=pmu_t[:], lhsT=ones[0:1, :], rhs=murs[:, 0:512],
                                 start=True, stop=True)
                prs_t = ps_a.tile([P, 512], F32, tag="ps_a")
                nc.tensor.matmul(out=prs_t[:], lhsT=ones[0:1, :], rhs=murs[:, 512:1024],
                                 start=True, stop=True)
                pmu, prs = pmu_t[:], prs_t[:]
            for dt in range(DT):
                scr = lp3.tile([P, 512], MDT, tag="ln_scr", bufs=2)
                nc.vector.tensor_sub(out=scr[:, :w], in0=fn(dt), in1=pmu)
                nc.vector.tensor_mul(out=y[:, dt, ds(col0, w)], in0=scr[:, :w], in1=prs)

    # ------------------------------------------------ transformer layers
    # Each layer is split into token-halves: the second half (tokens 256-511)
    # finishes first — through attention qB=1, O-proj, MLP, residual — and its
    # x is exported + AllGathered mid-layer. The gather-dependent first half
    # (halo LN, K/V halo, qB=0) of the NEXT layer then trails the export by a
    # full half-layer of independent work, hiding collective latency + skew.
    _knl = int(os.environ.get("KNL", NL))
    prev_agout = None

    def emit_halo(hl, y_t, kr_t, vt01, wk_t, wv_t):
        """LN + K/V projections for layer hl's halo tokens (reads xh).
        Hoisted into the previous layer's tail: its DVE chain overlaps the
        MLP matmul stream, and its matmuls fill the next layer's LN1 chain."""
        emit_ln(srcs=[(lambda dt: xh[:, dt, :], 0, HALF)], y=y_t)
        for do in range(DT):
            pk = ps_a.tile([P, HALF], F32, tag="ps_a")
            for dt in range(DT):
                nc.tensor.matmul(out=pk[:], lhsT=wk_t[:, dt, ts(do, P)],
                                 rhs=y_t[:, dt, ds(0, HALF)],
                                 start=(dt == 0), stop=(dt == DT - 1))
            nc.scalar.activation(kr_t[:, do, ds(0, HALF)], pk[:], AF.Identity,
                                 bias=bkt[:, hl, do:do + 1], scale=1.0)
        for t in range(2):
            pv = ps_a.tile([P, D], F32, tag="ps_a")
            for dt in range(DT):
                nc.tensor.matmul(out=pv[:], lhsT=y_t[:, dt, ts(t, P)], rhs=wv_t[:, dt, :],
                                 start=(dt == 0), stop=(dt == DT - 1))
            vtv = vt01[t][:].rearrange("p (h c) -> p h c", c=DH + 1)
            nc.vector.tensor_add(
                out=vtv[:, :, 0:DH],
                in0=pv[:].rearrange("p (h c) -> p h c", c=DH),
                in1=bvt[:, hl, :].rearrange("p (h c) -> p h c", c=DH))
            nc.vector.tensor_copy(out=vtv[:, :, DH:DH + 1], in_=ones[:, 0:H])

    def alloc_head_tiles():
        y_t = lp.tile([P, DT, W], MDT, tag="y", bufs=2, name="y")
        kr_t = lp.tile([P, DT, W], MDT, tag="kr", bufs=2, name="kr")
        vt01 = [lp.tile([P, H * (DH + 1)], MDT, tag=f"vt{t}", bufs=2, name=f"vt{t}")
                for t in range(2)]
        return y_t, kr_t, vt01

    for l in range(_knl):
        wq_r, wk_r, wv_r, wo_r = wcur["wq"], wcur["wk"], wcur["wv"], wcur["wo"]
        w1r, w2r = mcur
        y, kr, vt01 = alloc_head_tiles()
        vt = vt01 + [lp.tile([P, H * (DH + 1)], MDT, tag=f"vtb{t}", name=f"vt{t}")
                     for t in range(2, W // P)]

        # import the halo gathered during the previous layer (gpsimd; waits
        # only on its data, placed before this layer's collective in-queue)
        if l == 0:
            emit_halo(0, y, kr, vt01, wk_r, wv_r)
        else:
            for dt in range(DT):
                nc.gpsimd.indirect_dma_start(
                    out=xh[:, dt, :], out_offset=None, in_=prev_agout[:],
                    in_offset=bass.IndirectOffsetOnAxis(ap=hoffs[:, dt:dt + 1], axis=0))

        # LN1 on own tokens (halo columns were filled last layer)
        emit_ln(srcs=[(lambda dt: x[:, dt, :], HALF, CHUNK)], y=y)

        # --- Q projection (own tokens only)
        qr = lp.tile([P, DT, CHUNK], MDT, tag="qr")
        for do in range(DT):
            pq = ps_a.tile([P, CHUNK], F32, tag="ps_a")
            for dt in range(DT):
                nc.tensor.matmul(out=pq[:], lhsT=wq_r[:, dt, ts(do, P)],
                                 rhs=y[:, dt, ds(HALF, CHUNK)],
                                 start=(dt == 0), stop=(dt == DT - 1))
            nc.scalar.activation(qr[:, do, :], pq[:], AF.Identity,
                                 bias=bqt[:, l, do:do + 1], scale=1.0)

        # --- K/V projections, own tokens
        for do in range(DT):
            pk = ps_a.tile([P, CHUNK], F32, tag="ps_a")
            for dt in range(DT):
                nc.tensor.matmul(out=pk[:], lhsT=wk_r[:, dt, ts(do, P)],
                                 rhs=y[:, dt, ds(HALF, CHUNK)],
                                 start=(dt == 0), stop=(dt == DT - 1))
            nc.scalar.activation(kr[:, do, ds(HALF, CHUNK)], pk[:], AF.Identity,
                                 bias=bkt[:, l, do:do + 1], scale=1.0)

        for t in range(2, W // P):
            pv = ps_a.tile([P, D], F32, tag="ps_a")
            for dt in range(DT):
                nc.tensor.matmul(out=pv[:], lhsT=y[:, dt, ts(t, P)], rhs=wv_r[:, dt, :],
                                 start=(dt == 0), stop=(dt == DT - 1))
            vtv = vt[t][:].rearrange("p (h c) -> p h c", c=DH + 1)
            nc.vector.tensor_add(
                out=vtv[:, :, 0:DH],
                in0=pv[:].rearrange("p (h c) -> p h c", c=DH),
                in1=bvt[:, l, :].rearrange("p (h c) -> p h c", c=DH))
            nc.vector.tensor_copy(out=vtv[:, :, DH:DH + 1], in_=ones[:, 0:H])

        # --- sliding-window attention
        attr = lp.tile([P, DT, CHUNK], MDT, tag="attr")
        ej_keep = {}

        def emit_ej(h, kt, keep):
            _, q0, w = KTW[kt]
            r0, dto = (h % 2) * DH, h // 2
            pscore = ps_b.tile([P, 512], F32, tag="ps_b")
            nc.tensor.matmul(out=pscore[:, 0:w],
                             lhsT=kr[ds(r0, DH), dto, ts(kt, P)],
                             rhs=qr[ds(r0, DH), dto, ds(q0, w)],
                             start=True, stop=True)
            ej = lp3.tile([P, 512], MDT, tag="ej_keep" if keep else "ej_tmp",
                          bufs=2 * H if keep else 8, name="ej")
            nc.scalar.activation(ej[:, 0:w], pscore[:, 0:w], AF.Exp,
                                 bias=negb[:], scale=SCALE)
            nc.vector.tensor_mul(out=ej[:, 0:w], in0=ej[:, 0:w], in1=maskt[:, kt, 0:w])
            return ej

        def emit_qblock(h, qB, ejs):
            r0, dto = (h % 2) * DH, h // 2
            pa = ps_c.tile([DH + 1, 256], F32, tag="ps_c")
            for i, kt in enumerate(range(qB * 2, qB * 2 + 4)):
                c0 = qB * 256 - KTW[kt][1]
                nc.tensor.matmul(out=pa[:], lhsT=vt[kt][:, ds(h * (DH + 1), DH + 1)],
                                 rhs=ejs[kt][:, ds(c0, 256)],
                                 start=(i == 0), stop=(i == 3))
            # reciprocal_approx_* misreads PSUM at base_partition!=0 — stage
            # the denominator row (partition 64) to partition-0 SBUF first
            srow = lp3.tile([1, 256], F32, tag="srow", bufs=4)
            nc.vector.tensor_copy(out=srow[:], in_=pa[DH:DH + 1, :])
            rr = lp3.tile([1, 256], F32, tag="rr", bufs=4)
            nc.vector.reciprocal_approx_fast(out=rr[:], in_=srow[:])
            pbc = ps_a.tile([DH, 256], F32, tag="ps_a")
            nc.tensor.matmul(out=pbc[:], lhsT=ones_f[0:1, 0:DH], rhs=rr[:],
                             start=True, stop=True)
            dst = attr[ds(r0, DH), dto, ds(qB * 256, 256)]
            bcs = lp3.tile([DH, 256], MDT, tag="bcs", bufs=4)
            nc.vector.tensor_copy(out=bcs[:], in_=pbc[:])
            nc.vector.tensor_mul(out=dst, in0=pa[0:DH, :], in1=bcs[:])

        # --- helpers for token-half processing (c0 = 0 or 256)
        def emit_oproj_half(c0):
            for do in range(DT):
                po = ps_a.tile([P, HALF], F32, tag="ps_a")
                for dt in range(DT):
                    nc.tensor.matmul(out=po[:], lhsT=wo_r[:, dt, ts(do, P)],
                                     rhs=attr[:, dt, ds(c0, HALF)],
                                     start=(dt == 0), stop=(dt == DT - 1))
                nc.vector.tensor_add(out=x[:, do, ds(c0, HALF)],
                                     in0=x[:, do, ds(c0, HALF)], in1=po[:])

        def emit_mlp_half(c0):
            emit_ln(srcs=[(lambda dt: x[:, dt, ds(c0, HALF)], c0, HALF)], y=y2)
            pb = [ps_b.tile([P, HALF], F32, tag="ps_b", name=f"pb{i}") for i in range(DT)]

            def emit_mlp2(m, hm):
                for do in range(DT):
                    nc.tensor.matmul(out=pb[do][:], lhsT=w2r[:, m, ts(do, P)],
                                     rhs=hm[:], start=(m == 0), stop=(m == MT - 1))

            hist = []
            for m in range(MT):
                p1 = ps_a.tile([P, HALF], F32, tag="ps_a")
                for dt in range(DT):
                    nc.tensor.matmul(out=p1[:], lhsT=w1r[:, dt, ts(m, P)],
                                     rhs=y2[:, dt, ds(c0, HALF)],
                                     start=(dt == 0), stop=(dt == DT - 1))
                hm = lp3.tile([P, HALF], MDT, tag="hm", bufs=4)
                nc.scalar.activation(hm[:], p1[:], AF.Gelu_apprx_tanh,
                                     bias=b1t[:, l, m:m + 1], scale=1.0)
                hist.append((m, hm))
                if len(hist) > 2:
                    emit_mlp2(*hist.pop(0))
            for mm_, hh_ in hist:
                emit_mlp2(mm_, hh_)
            for do in range(DT):
                nc.vector.scalar_tensor_tensor(
                    out=x[:, do, ds(c0, HALF)], in0=pb[do][:],
                    scalar=b2t[:, l, do:do + 1], in1=x[:, do, ds(c0, HALF)],
                    op0=OP.add, op1=OP.add)

        y2 = lp.tile([P, DT, CHUNK], MDT, tag="y2")

        # phase 1: own-key work for all heads (kt 2..5, qB=1), emitted in
        # waves of 4 heads: all score/exp chains first, then the AV blocks,
        # so the PE sees dense back-to-back matmuls.
        for wave in (range(0, 4), range(4, 8)):
            ejsw = {}
            for h in wave:
                ejsw[h] = {kt: emit_ej(h, kt, kt in (2, 3)) for kt in (2, 3, 4, 5)}
                ej_keep[h] = {kt: ejsw[h][kt] for kt in (2, 3)}
            for h in wave:
                emit_qblock(h, 1, ejsw[h])

        # second token-half completes through its residual, then exports
        emit_oproj_half(HALF)
        emit_mlp_half(HALF)
        if l < NL - 1:
            agin = drp.tile([D, HALF], F32R, tag=f"agin{l}")
            agout = drp.tile([len(GROUPS[0]) * D, HALF], F32R, tag=f"agout{l}")
            nc.sync.dma_start(out=agin[:].rearrange("(t p) m -> p t m", p=P),
                              in_=x[:, :, ds(HALF, HALF)])
            nc.gpsimd.collective_compute(
                "AllGather", OP.bypass, replica_groups=GROUPS,
                ins=[agin.opt()], outs=[agout.opt()])
            prev_agout = agout

        # halo: LN1 on xh + K/V halo columns (consumes prev layer's gather)
        if l > 0:
            emit_halo(l, y, kr, vt01, wk_r, wv_r)

        # phase 2: halo-key work (kt 0,1 + kt 2,3 kept from phase 1; qB=0)
        for wave in (range(0, 4), range(4, 8)):
            ejsw = {}
            for h in wave:
                ejsw[h] = dict(ej_keep[h])
                for kt in (0, 1):
                    ejsw[h][kt] = emit_ej(h, kt, False)
            for h in wave:
                emit_qblock(h, 0, ejsw[h])

        if l == 0 and "ydump" in io:
            for nm_t, src_t in (("ydump", y), ("qdump", qr), ("kdump", kr), ("adump", attr)):
                nc.sync.dma_start(out=io[nm_t].ap().rearrange("(t p) m -> p t m", p=P),
                                  in_=src_t[:])

        # prefetch next layer's attention weights (double-buffered pool)
        if l + 1 < _knl:
            wcur = load_qkvo(l + 1)

        # first token-half completes
        emit_oproj_half(0)
        emit_mlp_half(0)

        if l + 1 < _knl:
            mcur = load_mlp(l + 1)
        dump_x(l + 1)

    # ------------------------------------------------ final LN + logits
    # Token-sharded final: each core computes the FULL vocab for its own 512
    # tokens from the local yf — no collective, no cross-core skew exposure.
    # w_out (32MB, shared across cores) streams through a small rotating pool;
    # the first few tiles load during the last layer.
    emit_ln(srcs=[(lambda dt: x[:, dt, :], 0, CHUNK)], y=yf)

    lp3.release()
    lp.release()

    FBLK = 1024  # vocab columns per streamed weight tile (1KB DMA segments)
    with tc.tile_pool(name="ftrans", bufs=3) as ftp, \
         tc.tile_pool(name="fout", bufs=6) as fop:
        vb0 = 0
        while vb0 < V:
            vbw = min(FBLK, V - vb0)
            fwr = ftp.tile([P, DT, FBLK], MDT, tag="fwr")
            nc.sync.dma_start(out=fwr[:, :, 0:vbw],
                              in_=io["w_out_sl"].ap()[:, ds(vb0, vbw)]
                              .rearrange("(t p) m -> p t m", p=P))
            for vi in range(vbw // P):
                v_i = vb0 // P + vi
                pf = ps_a.tile([P, CHUNK], F32, tag="ps_a")
                for dt in range(DT):
                    nc.tensor.matmul(out=pf[:], lhsT=fwr[:, dt, ts(vi, P)],
                                     rhs=yf[:, dt, :], start=(dt == 0), stop=(dt == DT - 1))
                ot = fop.tile([P, CHUNK], F16, tag="fot")
                if v_i % 2 == 0:
                    nc.scalar.activation(ot[:], pf[:], AF.Identity,
                                         bias=bot[:, v_i:v_i + 1], scale=1.0)
                else:
                    nc.vector.tensor_scalar_add(out=ot[:], in0=pf[:],
                                                scalar1=bot[:, v_i:v_i + 1])
                nc.sync.dma_start(out=io["out"].ap()[ts(v_i, P), :], in_=ot[:])
            vb0 += vbw

    drp.release()
    ps_c.release()
    ps_b.release()
    ps_a.release()
    wmlp.release()
    wqk.release()
    xpool.release()
    cpool.release()


# ================================================================ host side
def _pe_table():
    pos = np.arange(S, dtype=np.float32)[:, None]
    div = np.exp(np.arange(0, D, 2, dtype=np.float32) * -(np.log(10000.0) / D))
    pe = np.zeros((S, D), dtype=np.float32)
    pe[:, 0::2] = np.sin(pos * div)
    pe[:, 1::2] = np.cos(pos * div)
    return pe


def _in_maps(inputs):
    inp = np.asarray(inputs["inputs"]).astype(np.int32)
    ids = np.pad(inp, ((0, 0), (1, 0)))[:, :-1].astype(np.int32)
    pe = _pe_table()

    f32 = lambda k: np.asarray(inputs[k], dtype=np.float32)
    ln1_s, ln1_b = f32("ln1_s"), f32("ln1_b")
    ln2_s, ln2_b = f32("ln2_s"), f32("ln2_b")
    lnf_s, lnf_b = f32("lnf_s").reshape(D), f32("lnf_b").reshape(D)
    wq, wk, wv, wo = f32("wq"), f32("wk"), f32("wv"), f32("wo")
    w1, w2 = f32("w1"), f32("w2")
    b1, b2 = f32("b1"), f32("b2")
    wout, bout = f32("w_out"), f32("b_out")

    # fold LN affine into the downstream projections
    wq_f = wq * ln1_s[:, :, None]
    wk_f = wk * ln1_s[:, :, None]
    wv_f = wv * ln1_s[:, :, None]
    w1_f = w1 * ln2_s[:, :, None]
    bq = np.einsum("ld,ldm->lm", ln1_b, wq)
    bk = np.einsum("ld,ldm->lm", ln1_b, wk)
    bv = np.einsum("ld,ldm->lm", ln1_b, wv)
    b1_f = b1 + np.einsum("ld,ldm->lm", ln2_b, w1)
    wout_f = wout * lnf_s[:, None]
    bout_f = bout + lnf_b @ wout

    shared = {
        "embed": np.ascontiguousarray(f32("embed")),
        "b1": b1_f, "b2": b2, "bq": bq, "bk": bk,
        "bv": np.ascontiguousarray(np.broadcast_to(bv[None], (P, NL, D)).astype(np.float16)),
        "wq": wq_f.astype(np.float16), "wk": wk_f.astype(np.float16),
        "wv": wv_f.astype(np.float16), "wo": wo.astype(np.float16),
        "w1": w1_f.astype(np.float16), "w2": w2.astype(np.float16),
    }
    shared["w_out_sl"] = np.ascontiguousarray(wout_f.astype(np.float16))
    shared["b_out_sl"] = np.ascontiguousarray(bout_f.reshape(1, V).astype(np.float32))
    shared = {k: np.ascontiguousarray(v) for k, v in shared.items()}

    maps = []
    for c in range(NCORES):
        b, ch = divmod(c, NCORES // B)
        t0 = ch * CHUNK
        lo = t0 - HALF
        ids768 = np.zeros(W, np.int32)
        pe768 = np.zeros((W, D), np.float32)
        s0 = max(0, lo)
        ids768[s0 - lo:] = ids[b, s0:t0 + CHUNK]
        pe768[s0 - lo:] = pe[s0:t0 + CHUNK]
        # per-key-tile masks: [6, 128, 512] f16
        m = np.zeros((6, P, 512), np.float16)
        for kt, q0, w in KTW:
            uk = kt * P + np.arange(P)[:, None]
            q = q0 + np.arange(w)[None, :]
            dqk = (HALF + q) - uk
            ok = (dqk >= 0) & (dqk <= HALF)
            if ch == 0:
                ok = ok & ((lo + uk) >= 0)
            m[kt, :, :w] = ok.astype(np.float16)
        src = ch - 1 if ch > 0 else 0
        hoffs = (src * D + np.arange(DT)[None, :] * P
                 + np.arange(P)[:, None]).astype(np.int32)
        mp = dict(shared)
        mp.update(
            idx_in=np.ascontiguousarray(ids768.reshape(W // P, P).T),
            pe_dm=np.ascontiguousarray(pe768.T),
            masks=m, halo_offs=hoffs)
        maps.append(mp)
    return maps


def _assemble(res):
    full = np.empty((NTOK, V), np.float32)
    for c in range(NCORES):
        full[c * CHUNK:(c + 1) * CHUNK, :] = \
            np.asarray(res[c]["logits_vm"], dtype=np.float32).T
    return full.reshape(B, S, V)


def kernel(**inputs):
    nc = _CACHE.get("nc")
    if nc is None:
        nc = _build()
        _CACHE["nc"] = nc
    maps = _in_maps(inputs)
    res = run_bass_kernel_spmd(nc, maps, list(range(NCORES))).results
    return _assemble(res)
